# revision 1
# baseline (speedup 1.0000x reference)
"""Trainium2 Bass kernel for a full causal MHA layer (B=2, T=2048, C=2048, H=16,
partial RoPE on first 64 dims of each 128-dim head).

Sharding over 8 cores: core c handles batch b=c//4 and heads [4g, 4g+4), g=c%4
(tensor-parallel over heads x data-parallel over batch). Each core:
  phase 1: q/k/v projections (fp32r matmuls), bias, partial RoPE on q/k,
           spill qT/kT [d,t] and v [t,m] to DRAM scratch
  phase 2: per (i-chunk, head), causal attention in transposed layout:
           scoresT[j,i] -> exp (no max subtraction; causal logits peak ~9.5)
           -> causal mask on diagonal tiles -> out accumulation outT[d,i]
           with row-sums via a ones-matmul -> normalize -> attnT[m,t]
  phase 3: output projection partial outT[c,t], per t-chunk (overlaps ph2 tail)
Host: slices inputs per core, sums the 4 TP partials per batch, adds bo.
"""

import math

import numpy as np

import concourse.bass as bass
import concourse.mybir as mybir
import concourse.tile as tile
from concourse import bacc
from concourse.bass_utils import run_bass_kernel_spmd

F32 = mybir.dt.float32
F32R = mybir.dt.float32r

B, T, C = 2, 2048, 2048
H = 16
HS = 128
ROT = 64
HALF = 32
BASE = 10000.0

N_CORES = 8
TPG = 4                # TP group size (heads split)
H_LOC = H // TPG       # 4 heads per core
M = H_LOC * HS         # 512 local head-dim columns
SCALE = 1.0 / math.sqrt(HS)

P = 128
NT = T // 512          # 4 t-chunks of 512
CT = C // P            # 16 contraction tiles
JT = T // P            # 16 key tiles per head

_NC_CACHE = {}


def _build(phases=(1, 2, 3)):
    nc = bacc.Bacc(None, target_bir_lowering=False)

    xT = nc.declare_dram_parameter("xT", [C, T], F32R, isOutput=False)
    wqT = nc.declare_dram_parameter("wqT", [C, M], F32R, isOutput=False)
    wkT = nc.declare_dram_parameter("wkT", [C, M], F32R, isOutput=False)
    wvT = nc.declare_dram_parameter("wvT", [C, M], F32R, isOutput=False)
    woT = nc.declare_dram_parameter("woT", [M, C], F32R, isOutput=False)
    bqc = nc.declare_dram_parameter("bqc", [P, H_LOC], F32, isOutput=False)
    bkc = nc.declare_dram_parameter("bkc", [P, H_LOC], F32, isOutput=False)
    bvr = nc.declare_dram_parameter("bvr", [1, M], F32R, isOutput=False)
    cosT = nc.declare_dram_parameter("cosT", [ROT, T], F32, isOutput=False)
    ones1_d = nc.declare_dram_parameter("ones1", [1, P], F32R, isOutput=False)
    ones128_d = nc.declare_dram_parameter("ones128", [P, 1], F32R, isOutput=False)
    nsT = nc.declare_dram_parameter("nsT", [ROT, T], F32, isOutput=False)
    outT = nc.declare_dram_parameter("outT", [C, T], F32, isOutput=True)

    qt_d = nc.dram_tensor("qt_scratch", [H_LOC, P, T], F32R)
    v_d = nc.dram_tensor("v_scratch", [T, M], F32R)

    with tile.TileContext(nc) as tc, \
         tc.tile_pool(name="const", bufs=1) as const:
        cos_sb = const.tile([ROT, T], F32, tag="cos")
        ns_sb = const.tile([ROT, T], F32, tag="ns")
        bq_sb = const.tile([P, H_LOC], F32, tag="bq")
        bk_sb = const.tile([P, H_LOC], F32, tag="bk")
        bv_sb = const.tile([1, M], F32R, tag="bv")
        ones1 = const.tile([1, P], F32R, tag="ones1")
        ones128 = const.tile([P, 1], F32R, tag="ones128")
        bvb_sb = const.tile([P, M], F32R, tag="bvb")
        k_res = const.tile([P, H_LOC, T], F32R, tag="kres")
        nc.sync.dma_start(out=cos_sb[:], in_=cosT[:])
        nc.sync.dma_start(out=ns_sb[:], in_=nsT[:])
        nc.sync.dma_start(out=bq_sb[:], in_=bqc[:])
        nc.sync.dma_start(out=bk_sb[:], in_=bkc[:])
        nc.sync.dma_start(out=bv_sb[:], in_=bvr[:])
        nc.sync.dma_start(out=ones1[:], in_=ones1_d[:])
        nc.sync.dma_start(out=ones128[:], in_=ones128_d[:])

        # ---------------- phase 1: projections ----------------
        if 1 in phases:
         with tc.tile_pool(name="p1w", bufs=CT) as wpool, \
             tc.tile_pool(name="p1x", bufs=16) as xpool, \
             tc.tile_pool(name="p1e", bufs=2) as epool, \
             tc.tile_pool(name="p1r", bufs=2) as rpool, \
             tc.tile_pool(name="p1psqk", bufs=6, space="PSUM") as psqkpool, \
             tc.tile_pool(name="p1psv", bufs=2, space="PSUM") as psvpool:
            # broadcast bv across 128 partitions once via stride-0 DMA read
            nc.sync.dma_start(out=bvb_sb[:], in_=bvr[0:1, :].to_broadcast([P, M]))

            wq_t = [wpool.tile([P, M], F32R, tag="wq", name=f"wq{i}") for i in range(CT)]
            wk_t = [wpool.tile([P, M], F32R, tag="wk", name=f"wk{i}") for i in range(CT)]
            wv_t = [wpool.tile([P, M], F32R, tag="wv", name=f"wv{i}") for i in range(CT)]
            # first x chunk before weights so the first matmul group starts early
            x0_t = [xpool.tile([P, 512], F32R, tag="x", name=f"x0_{i}")
                    for i in range(CT)]
            for ct in range(CT):
                nc.sync.dma_start(out=x0_t[ct][:], in_=xT[ct * P:(ct + 1) * P, 0:512])
                nc.sync.dma_start(out=wq_t[ct][:], in_=wqT[ct * P:(ct + 1) * P, :])
            x1_t = [xpool.tile([P, 512], F32R, tag="x", name=f"x1_{i}")
                    for i in range(CT)]
            for ct in range(CT):
                nc.sync.dma_start(out=wk_t[ct][:], in_=wkT[ct * P:(ct + 1) * P, :])
                nc.sync.dma_start(out=x1_t[ct][:],
                                  in_=xT[ct * P:(ct + 1) * P, 512:1024])
                nc.sync.dma_start(out=wv_t[ct][:], in_=wvT[ct * P:(ct + 1) * P, :])

            for tch in range(NT):
                ts0 = tch * 512
                if tch == 0:
                    x_t = x0_t
                elif tch == 1:
                    x_t = x1_t
                else:
                    x_t = [xpool.tile([P, 512], F32R, tag="x", name=f"x{tch}_{i}")
                           for i in range(CT)]
                    for ct in range(CT):
                        nc.sync.dma_start(
                            out=x_t[ct][:],
                            in_=xT[ct * P:(ct + 1) * P, ts0:ts0 + 512])

                for proj, w_t, b_sb in (("q", wq_t, bq_sb), ("k", wk_t, bk_sb)):
                    for mt in range(H_LOC):
                        ps = psqkpool.tile([P, 512], F32, tag="psqk")
                        for ct in range(CT):
                            nc.tensor.matmul(
                                ps[:],
                                lhsT=w_t[ct][:, mt * P:(mt + 1) * P],
                                rhs=x_t[ct][:],
                                start=(ct == 0), stop=(ct == CT - 1))
                        if proj == "q":
                            # full biased evict, rope rows 0..63, spill to DRAM
                            qtmp = epool.tile([P, 512], F32R, tag="qtmp")
                            nc.scalar.activation(
                                qtmp[:], ps[:],
                                mybir.ActivationFunctionType.Identity,
                                bias=b_sb[:, mt:mt + 1], scale=1.0)
                            qsh = rpool.tile([ROT, 512], F32R, tag="qsh")
                            nc.sync.dma_start(out=qsh[0:HALF], in_=qtmp[HALF:ROT])
                            nc.sync.dma_start(out=qsh[HALF:ROT], in_=qtmp[0:HALF])
                            qrot = rpool.tile([ROT, 512], F32, tag="qrot")
                            nc.vector.tensor_tensor(
                                qrot[:], qsh[:], ns_sb[:, ts0:ts0 + 512],
                                mybir.AluOpType.mult)
                            tcos = rpool.tile([ROT, 512], F32, tag="tcos")
                            nc.vector.tensor_tensor(
                                tcos[:], qtmp[0:ROT], cos_sb[:, ts0:ts0 + 512],
                                mybir.AluOpType.mult)
                            nc.vector.tensor_tensor(
                                qtmp[0:ROT], tcos[:], qrot[:],
                                mybir.AluOpType.add)
                            nc.sync.dma_start(
                                out=qt_d[mt, :, ts0:ts0 + 512], in_=qtmp[:])
                        else:
                            # k stays in SBUF: rows 64..127 straight into k_res,
                            # rows 0..63 biased to tmp, rope, write into k_res
                            nc.scalar.activation(
                                k_res[ROT:P, mt, ts0:ts0 + 512], ps[ROT:P],
                                mybir.ActivationFunctionType.Identity,
                                bias=b_sb[ROT:P, mt:mt + 1], scale=1.0)
                            ktmp = epool.tile([ROT, 512], F32R, tag="ktmp")
                            nc.scalar.activation(
                                ktmp[:], ps[0:ROT],
                                mybir.ActivationFunctionType.Identity,
                                bias=b_sb[0:ROT, mt:mt + 1], scale=1.0)
                            ksh = rpool.tile([ROT, 512], F32R, tag="qsh")
                            nc.sync.dma_start(out=ksh[0:HALF], in_=ktmp[HALF:ROT])
                            nc.sync.dma_start(out=ksh[HALF:ROT], in_=ktmp[0:HALF])
                            krot = rpool.tile([ROT, 512], F32, tag="qrot")
                            nc.vector.tensor_tensor(
                                krot[:], ksh[:], ns_sb[:, ts0:ts0 + 512],
                                mybir.AluOpType.mult)
                            kcos = rpool.tile([ROT, 512], F32, tag="tcos")
                            nc.vector.tensor_tensor(
                                kcos[:], ktmp[:], cos_sb[:, ts0:ts0 + 512],
                                mybir.AluOpType.mult)
                            nc.vector.tensor_tensor(
                                k_res[0:ROT, mt, ts0:ts0 + 512], kcos[:], krot[:],
                                mybir.AluOpType.add)

                # v: [t_tile, m] layout, bias via broadcast add
                for tt in range(4):
                    ps = psvpool.tile([P, M], F32, tag="psv")
                    for ct in range(CT):
                        nc.tensor.matmul(
                            ps[:],
                            lhsT=x_t[ct][:, tt * P:(tt + 1) * P],
                            rhs=wv_t[ct][:],
                            start=(ct == 0), stop=(ct == CT - 1))
                    vtmp = epool.tile([P, M], F32R, tag="vtmp")
                    nc.vector.tensor_tensor(
                        vtmp[:], ps[:], bvb_sb[:], mybir.AluOpType.add)
                    t0 = ts0 + tt * P
                    nc.sync.dma_start(out=v_d[t0:t0 + P, :], in_=vtmp[:])

        # ---------------- phases 2+3 ----------------
        if 2 in phases:
         with tc.tile_pool(name="attn", bufs=1) as apool, \
             tc.tile_pool(name="p2v", bufs=1) as vpool, \
             tc.tile_pool(name="p2q", bufs=6) as qpool, \
             tc.tile_pool(name="p2e", bufs=6) as expool, \
             tc.tile_pool(name="p2d", bufs=2) as denpool, \
             tc.tile_pool(name="p3e", bufs=4) as oepool, \
             tc.tile_pool(name="p3w", bufs=H_LOC) as wopool, \
             tc.tile_pool(name="p2ps", bufs=3, space="PSUM") as ps2, \
             tc.tile_pool(name="p2psd", bufs=2, space="PSUM") as ps2d, \
             tc.tile_pool(name="p2psb", bufs=1, space="PSUM") as ps2b, \
             tc.tile_pool(name="p2pso", bufs=2, space="PSUM") as ps2o:
            attn_c = [apool.tile([P, H_LOC, 512], F32R, tag=f"attnT{i}",
                                 name=f"attn{i}") for i in range(NT)]
            v_r = v_d[:].rearrange("(jt p) m -> p jt m", p=P)
            vh_c = [[vpool.tile([P, 4, HS], F32R, tag=f"vh{h}_{jc}",
                                name=f"vh{h}_{jc}") for jc in range(NT)]
                    for h in range(H_LOC)]
            def load_vh(jc):
                for h in range(H_LOC):
                    nc.sync.dma_start(
                        out=vh_c[h][jc][:],
                        in_=v_r[:, 4 * jc:4 * jc + 4, h * HS:(h + 1) * HS])

            load_vh(0)

            wo_t = [wopool.tile([P, C], F32R, tag="wo", name=f"wo{i}")
                    for i in range(H_LOC)]

            for ic in range(NT):
                i0 = ic * 512
                njt = 4 * ic + 4
                qc_t = []
                for h in range(H_LOC):
                    qc = qpool.tile([P, 512], F32R, tag="qc", name=f"qc{ic}_{h}")
                    nc.sync.dma_start(out=qc[:], in_=qt_d[h, :, i0:i0 + 512])
                    qc_t.append(qc)
                if ic + 1 < NT:
                    load_vh(ic + 1)
                for h in range(H_LOC):
                    qc = qc_t[h]
                    ps_out = ps2o.tile([P, 512], F32, tag="psout")
                    ps_d = ps2d.tile([1, 512], F32, tag="psd")
                    for jt in range(njt):
                        ps_s = ps2.tile([P, 512], F32, tag="pss")
                        nc.tensor.matmul(
                            ps_s[:],
                            lhsT=k_res[:, h, jt * P:(jt + 1) * P],
                            rhs=qc[:],
                            start=True, stop=True)
                        ex = expool.tile([P, 512], F32R, tag="ex")
                        nc.scalar.activation(
                            ex[:], ps_s[:],
                            mybir.ActivationFunctionType.Exp, scale=SCALE)
                        if jt >= 4 * ic:
                            # keep where (i0 + il) - (jt*P + p) >= 0
                            nc.gpsimd.affine_select(
                                out=ex[:], in_=ex[:],
                                compare_op=mybir.AluOpType.is_ge,
                                fill=0.0,
                                base=i0 - jt * P,
                                channel_multiplier=-1,
                                pattern=[[1, 512]])
                        nc.tensor.matmul(
                            ps_out[:],
                            lhsT=vh_c[h][jt // 4][:, jt % 4, :],
                            rhs=ex[:],
                            start=(jt == 0), stop=(jt == njt - 1))
                        nc.tensor.matmul(
                            ps_d[:], lhsT=ones128[:], rhs=ex[:],
                            start=(jt == 0), stop=(jt == njt - 1))
                    # reciprocal straight from PSUM, rounded to fp32r
                    rrow = denpool.tile([1, 512], F32R, tag="rrow")
                    with nc.allow_low_precision(reason="softmax 1/den in fp32r"):
                        nc.vector.reciprocal(rrow[:], ps_d[:])
                    ps_b = ps2b.tile([P, 512], F32, tag="psb")
                    nc.tensor.matmul(ps_b[:], lhsT=ones1[:],
                                     rhs=rrow[:], start=True, stop=True)
                    rden = denpool.tile([P, 512], F32, tag="rden")
                    nc.vector.tensor_copy(out=rden[:], in_=ps_b[:])
                    nc.vector.tensor_tensor(
                        attn_c[ic][:, h, :], ps_out[:], rden[:],
                        mybir.AluOpType.mult)

                if ic == 0:
                    # deferred so boundary DMA bandwidth goes to vh/qc first
                    for mt in range(H_LOC):
                        nc.sync.dma_start(out=wo_t[mt][:],
                                          in_=woT[mt * P:(mt + 1) * P, :])
                # ---------------- phase 3 for this t-chunk ----------------
                if 3 in phases:
                    for co in range(CT):
                        ps = ps2.tile([P, 512], F32, tag="pss")
                        for mt in range(H_LOC):
                            nc.tensor.matmul(
                                ps[:],
                                lhsT=wo_t[mt][:, co * P:(co + 1) * P],
                                rhs=attn_c[ic][:, mt, :],
                                start=(mt == 0), stop=(mt == H_LOC - 1))
                        ot = oepool.tile([P, 512], F32, tag="ot")
                        nc.vector.tensor_copy(out=ot[:], in_=ps[:])
                        nc.sync.dma_start(
                            out=outT[co * P:(co + 1) * P, i0:i0 + 512],
                            in_=ot[:])

    nc.finalize()
    return nc


def get_nc(phases=(1, 2, 3)):
    if phases not in _NC_CACHE:
        _NC_CACHE[phases] = _build(phases)
    return _NC_CACHE[phases]


def _rope_tables():
    inv_freq = 1.0 / (BASE ** (np.arange(0, ROT, 2, dtype=np.float64) / ROT))
    freqs = np.arange(T, dtype=np.float64)[:, None] * inv_freq[None, :]  # [T, 32]
    cos_h = np.cos(freqs).T.astype(np.float32)   # [32, T]
    sin_h = np.sin(freqs).T.astype(np.float32)
    cosT = np.concatenate([cos_h, cos_h], axis=0)          # [64, T]
    nsT = np.concatenate([-sin_h, sin_h], axis=0)          # [64, T] signed sin
    return np.ascontiguousarray(cosT), np.ascontiguousarray(nsT)


def make_in_maps(x, Wq, bq, Wk, bk, Wv, bv, Wo, bo):
    cosT, nsT = _rope_tables()
    in_maps = []
    for c in range(N_CORES):
        b, g = divmod(c, TPG)
        ms = slice(g * M, (g + 1) * M)
        in_maps.append({
            "xT": np.ascontiguousarray(x[b].T),
            "wqT": np.ascontiguousarray(Wq[ms].T),
            "wkT": np.ascontiguousarray(Wk[ms].T),
            "wvT": np.ascontiguousarray(Wv[ms].T),
            "woT": np.ascontiguousarray(Wo[:, ms].T),
            "bqc": np.ascontiguousarray(bq[ms].reshape(H_LOC, P).T),
            "bkc": np.ascontiguousarray(bk[ms].reshape(H_LOC, P).T),
            "bvr": np.ascontiguousarray(bv[ms].reshape(1, M)),
            "cosT": cosT,
            "ones1": np.ones((1, P), np.float32),
            "ones128": np.ones((P, 1), np.float32),
            "nsT": nsT,
        })
    return in_maps


def assemble(results, bo):
    out = np.empty((B, T, C), dtype=np.float32)
    for b in range(B):
        acc = results[b * TPG]["outT"].astype(np.float32).copy()
        for g in range(1, TPG):
            acc += results[b * TPG + g]["outT"]
        out[b] = acc.T + bo[None, :]
    return out


def kernel(x, Wq, bq, Wk, bk, Wv, bv, Wo, bo):
    nc = get_nc()
    in_maps = make_in_maps(np.asarray(x, np.float32),
                           np.asarray(Wq, np.float32), np.asarray(bq, np.float32),
                           np.asarray(Wk, np.float32), np.asarray(bk, np.float32),
                           np.asarray(Wv, np.float32), np.asarray(bv, np.float32),
                           np.asarray(Wo, np.float32), np.asarray(bo, np.float32))
    res = run_bass_kernel_spmd(nc, in_maps, list(range(N_CORES)))
    return assemble(res.results, np.asarray(bo, np.float32))



# revision 70
# speedup vs baseline: 1.2360x; 1.2360x over previous
"""Trainium2 Bass kernel for a full causal MHA layer (B=2, T=2048, C=2048, H=16,
partial RoPE on first 64 dims of each 128-dim head).

Sharding over 8 cores: core c handles batch b=c//4 and heads [4g, 4g+4), g=c%4.

v3 design (fp8 hi/lo projections + fp16 attention, fully SBUF-resident):
  - x and Wq/Wk/Wv split host-side into e4m3 hi + e4m3 lo residuals; the three
    projections run as 3-term DoubleRow fp8 chains (hi@hi + lo@hi + hi@lo),
    25% fewer PE cycles than f32r at ~4e-3 relative error.
  - biases folded into each PSUM chain as a 1-partition DoubleRow matmul.
  - q/k path fp16: rot rows evicted to fp16 (DVE), RoPE on DVE (2x mode),
    pass rows evicted on GPSIMD; q_sb/k_res SBUF-resident [128,4,T] fp16.
  - scores/out/den matmuls in fp16 (1 cyc/row); exp on ACT into fp16 pair
    tiles [128,2,512]; causal masking via trimmed affine_selects on GPSIMD.
  - softmax: reciprocal (DVE) -> ones-matmul broadcast into the den pair
    tile's free plane -> copy to SBUF -> fused normalize into fp16 attn.
  - phase 3: fp16 matmuls; result DMA'd straight from PSUM to DRAM (f32).
  - per t-chunk interleave: proj(t) -> attention(ic=t) -> out-proj(ic=t).
Host: slices inputs per core, sums the 4 TP partials per batch, adds bo.
"""

import math

import numpy as np
import ml_dtypes

import concourse.bass as bass
import concourse.mybir as mybir
import concourse.tile as tile
from concourse import bacc
from concourse.bass_utils import run_bass_kernel_spmd

F32 = mybir.dt.float32
F32R = mybir.dt.float32r
FP16 = mybir.dt.float16
E4 = mybir.dt.float8e4
DR = mybir.MatmulPerfMode.DoubleRow

B, T, C = 2, 2048, 2048
H = 16
HS = 128
ROT = 64
HALF = 32
BASE = 10000.0

N_CORES = 8
TPG = 4                # TP group size (heads split)
H_LOC = H // TPG       # 4 heads per core
M = H_LOC * HS         # 512 local head-dim columns
SCALE = 1.0 / math.sqrt(HS)

P = 128
NT = T // 512          # 4 t-chunks of 512
KP = C // 256          # 8 DoubleRow contraction pair-tiles
JT = T // P            # 16 key tiles per head
WSCALE = 256.0         # fp8 weight pre-scale (keeps hi/lo residuals normal)

_NC_CACHE = {}


def _build(phases=(1, 2, 3), debug=False):
    nc = bacc.Bacc(None, target_bir_lowering=False)
    dbg = {}
    if debug:
        dbg["q"] = nc.declare_dram_parameter("qdbg", [P, H_LOC, T], FP16,
                                             isOutput=True)
        dbg["k"] = nc.declare_dram_parameter("kdbg", [P, H_LOC, T], FP16,
                                             isOutput=True)
        dbg["v"] = nc.declare_dram_parameter("vdbg", [P, JT, H_LOC, HS], FP16,
                                             isOutput=True)
        dbg["at"] = nc.declare_dram_parameter("atdbg", [NT, P, H_LOC, 512], FP16,
                                              isOutput=True)

    xhT = nc.declare_dram_parameter("xhT", [C, T], E4, isOutput=False)
    xlT = nc.declare_dram_parameter("xlT", [C, T], E4, isOutput=False)
    w_d = {}
    for w in ("wq", "wk", "wv"):
        for p_ in ("h", "l"):
            w_d[w + p_] = nc.declare_dram_parameter(
                w + p_, [C, M], E4, isOutput=False)
    woh = nc.declare_dram_parameter("woh", [M, C], E4, isOutput=False)
    wol = nc.declare_dram_parameter("wol", [M, C], E4, isOutput=False)
    bqc = nc.declare_dram_parameter("bqc", [P, H_LOC], F32, isOutput=False)
    bkc = nc.declare_dram_parameter("bkc", [P, H_LOC], F32, isOutput=False)
    bvp = nc.declare_dram_parameter("bvp", [1, 2, M], E4, isOutput=False)
    onesx = nc.declare_dram_parameter("onesx", [1, 2, 512], E4, isOutput=False)
    ones_dn = nc.declare_dram_parameter("ones_dn", [P, 1], FP16, isOutput=False)
    ones1_d = nc.declare_dram_parameter("ones1", [1, P], F32R, isOutput=False)
    cosT = nc.declare_dram_parameter("cosT", [ROT, T], FP16, isOutput=False)
    nsT = nc.declare_dram_parameter("nsT", [ROT, T], FP16, isOutput=False)
    outT = nc.declare_dram_parameter("outT", [C, T], FP16, isOutput=True)

    # DoubleRow pair views of the fp8 operands: contraction c = kp*256+ko*128+p
    xhpr = xhT[:].rearrange("(kp two p) t -> kp p two t", two=2, p=P)
    xlpr = xlT[:].rearrange("(kp two p) t -> kp p two t", two=2, p=P)
    wpr = {k: v[:].rearrange("(kp two p) m -> kp p two m", two=2, p=P)
           for k, v in w_d.items()}
    wohpr = woh[:].rearrange("(kp two p) c -> kp p two c", two=2, p=P)
    wolpr = wol[:].rearrange("(kp two p) c -> kp p two c", two=2, p=P)

    with tile.TileContext(nc) as tc, \
         tc.tile_pool(name="const", bufs=1) as const:
        cos_sb = const.tile([ROT, T], FP16, tag="cos")
        ns_sb = const.tile([ROT, T], FP16, tag="ns")
        bqc_sb = const.tile([P, H_LOC], F32, tag="bqc")
        bkc_sb = const.tile([P, H_LOC], F32, tag="bkc")
        bvp_sb = const.tile([1, 2, M], E4, tag="bvp")
        onesx_sb = const.tile([1, 2, 512], E4, tag="onesx")
        ones_dn_sb = const.tile([P, 1], FP16, tag="onesdn")
        ones1 = const.tile([1, P], F32R, tag="ones1")
        k_res = const.tile([P, H_LOC, T], FP16, tag="kres")
        q_sb = const.tile([P, H_LOC, T], FP16, tag="qsb")
        v_sb = const.tile([P, JT, H_LOC, HS], FP16, tag="vsb")
        w_t = {k: [const.tile([P, 2, M], E4, name=f"{k}{i}", tag=f"{k}{i}")
                   for i in range(KP)] for k in wpr}
        woh_t = [const.tile([P, 2, C], E4, name=f"woh{i}", tag=f"woh{i}")
                 for i in range(2)]
        wol_t = [const.tile([P, 2, C], E4, name=f"wol{i}", tag=f"wol{i}")
                 for i in range(2)]

        nc.sync.dma_start(out=cos_sb[:], in_=cosT[:])
        nc.sync.dma_start(out=ns_sb[:], in_=nsT[:])
        nc.sync.dma_start(out=bqc_sb[:], in_=bqc[:])
        nc.sync.dma_start(out=bkc_sb[:], in_=bkc[:])
        nc.sync.dma_start(out=bvp_sb[:], in_=bvp[:])
        nc.sync.dma_start(out=onesx_sb[:], in_=onesx[:])
        nc.sync.dma_start(out=ones_dn_sb[:], in_=ones_dn[:])
        nc.sync.dma_start(out=ones1[:], in_=ones1_d[:])

        with tc.tile_pool(name="px", bufs=24) as xpool, \
             tc.tile_pool(name="rope", bufs=4) as rpool, \
             tc.tile_pool(name="ex", bufs=3) as expool, \
             tc.tile_pool(name="den", bufs=2) as denpool, \
             tc.tile_pool(name="attnp", bufs=2) as apool, \
             tc.tile_pool(name="pair", bufs=3, space="PSUM") as pspair, \
             tc.tile_pool(name="pso", bufs=2, space="PSUM") as psout:

            # startup loads in chain consumption order: per-kp q weights and
            # chunk-0 x tiles interleaved, then k/v weights, wo last; later x
            # chunks drip in during phase1 so they never block shift DMAs
            x_t = {}
            prefetch = []

            def queue_x(tch):
                ts0 = tch * 512
                x_t[tch] = th = {}
                for nm, view in (("h", xhpr), ("l", xlpr)):
                    th[nm] = [xpool.tile([P, 2, 512], E4, tag="x",
                                         name=f"x{nm}{tch}_{i}")
                              for i in range(KP)]
                for kp in range(KP):
                    for nm, view in (("h", xhpr), ("l", xlpr)):
                        prefetch.append((th[nm][kp], view, kp, ts0))

            def drip(n):
                for _ in range(min(n, len(prefetch))):
                    tile_, view, kp, ts0 = prefetch.pop(0)
                    nc.sync.dma_start(out=tile_[:],
                                      in_=view[kp, :, :, ts0:ts0 + 512])

            queue_x(0)
            for kp in range(KP):
                drip(1)
                nc.sync.dma_start(out=w_t["wqh"][kp][:], in_=wpr["wqh"][kp])
                drip(1)
                nc.sync.dma_start(out=w_t["wql"][kp][:], in_=wpr["wql"][kp])
            for kp in range(KP):
                nc.sync.dma_start(out=w_t["wkh"][kp][:], in_=wpr["wkh"][kp])
                nc.sync.dma_start(out=w_t["wkl"][kp][:], in_=wpr["wkl"][kp])
            for kp in range(KP):
                nc.sync.dma_start(out=w_t["wvh"][kp][:], in_=wpr["wvh"][kp])
                nc.sync.dma_start(out=w_t["wvl"][kp][:], in_=wpr["wvl"][kp])
            for i in range(2):
                nc.sync.dma_start(out=woh_t[i][:], in_=wohpr[i])
                nc.sync.dma_start(out=wol_t[i][:], in_=wolpr[i])

            def phase1(tch):
                ts0 = tch * 512
                if tch + 1 < NT and (tch + 1) not in x_t:
                    queue_x(tch + 1)
                xh, xl = x_t[tch]["h"], x_t[tch]["l"]

                for proj, wn, b_sb in (("q", "wq", bqc_sb), ("k", "wk", bkc_sb)):
                    dst = q_sb if proj == "q" else k_res
                    wh, wl = w_t[wn + "h"], w_t[wn + "l"]
                    pt = None
                    for mt in range(H_LOC):
                        if mt % 2 == 0:
                            pt = pspair.tile([P, 2, 512], F32, tag="pp")
                        ps = pt[:, mt % 2, :]
                        ms = slice(mt * P, (mt + 1) * P)
                        nmm = 0
                        for kp in range(KP):
                            for wt_, xt_ in ((wh, xh), (wh, xl), (wl, xh)):
                                nmm += 1
                                nc.tensor.matmul(
                                    ps, lhsT=wt_[kp][:, :, ms], rhs=xt_[kp][:],
                                    start=(kp == 0 and wt_ is wh and xt_ is xh),
                                    stop=(nmm == 3 * KP), perf_mode=DR,
                                    skip_group_check=True)
                        # rot rows 0:64 -> fp16 tmp (bias + 1/WSCALE descale
                        # applied in the eviction), rope on DVE, write dst
                        qtmp = rpool.tile([ROT, 512], FP16, tag="qtmp")
                        with nc.allow_low_precision(reason="fp16 qk path"):
                            nc.scalar.activation(
                                qtmp[:], ps[0:ROT],
                                mybir.ActivationFunctionType.Identity,
                                bias=b_sb[0:ROT, mt:mt + 1], scale=1.0 / WSCALE)
                            # pass rows 64:128 straight to dst (ACT: gpsimd
                            # has no PSUM port)
                            nc.scalar.activation(
                                dst[ROT:P, mt, ts0:ts0 + 512], ps[ROT:P],
                                mybir.ActivationFunctionType.Identity,
                                bias=b_sb[ROT:P, mt:mt + 1], scale=1.0 / WSCALE)
                        qsh = rpool.tile([ROT, 512], FP16, tag="qsh")
                        nc.sync.dma_start(out=qsh[0:HALF], in_=qtmp[HALF:ROT])
                        nc.sync.dma_start(out=qsh[HALF:ROT], in_=qtmp[0:HALF])
                        t1 = rpool.tile([ROT, 512], FP16, tag="t1")
                        nc.vector.tensor_tensor(
                            t1[:], qtmp[:], cos_sb[:, ts0:ts0 + 512],
                            mybir.AluOpType.mult)
                        t2 = rpool.tile([ROT, 512], FP16, tag="t2")
                        nc.vector.tensor_tensor(
                            t2[:], qsh[:], ns_sb[:, ts0:ts0 + 512],
                            mybir.AluOpType.mult)
                        nc.vector.tensor_tensor(
                            dst[0:ROT, mt, ts0:ts0 + 512], t1[:], t2[:],
                            mybir.AluOpType.add)
                        drip(2)

                # v: [t_tile, m] layout
                wh, wl = w_t["wvh"], w_t["wvl"]
                pt = None
                for tt in range(4):
                    if tt % 2 == 0:
                        pt = pspair.tile([P, 2, 512], F32, tag="pp")
                    ps = pt[:, tt % 2, :]
                    ts_ = slice(tt * P, (tt + 1) * P)
                    for kp in range(KP):
                        for xt_, wt_ in ((xh, wh), (xl, wh), (xh, wl)):
                            nc.tensor.matmul(
                                ps, lhsT=xt_[kp][:, :, ts_], rhs=wt_[kp][:],
                                start=(kp == 0 and xt_ is xh and wt_ is wh),
                                stop=False, perf_mode=DR, skip_group_check=True)
                    nc.tensor.matmul(
                        ps, lhsT=onesx_sb[:, :, ts_], rhs=bvp_sb[:],
                        start=False, stop=True, perf_mode=DR,
                        skip_group_check=True)
                    jt = tch * 4 + tt
                    # v keeps the x256 weight scale; it is compensated by
                    # ones1 = 1/WSCALE in the softmax broadcast
                    with nc.allow_low_precision(reason="fp16 v"):
                        nc.vector.tensor_copy(out=v_sb[:, jt, :, :], in_=ps)
                    drip(2)

            def attention(ic):
                i0 = ic * 512
                npair = 2 * ic + 2
                at = apool.tile([P, H_LOC, 512], FP16, tag="attn")
                at_h = apool.tile([P, H_LOC, 512], E4, tag="attnh")
                at_l = apool.tile([P, H_LOC, 512], E4, tag="attnl")
                tail = []

                def flush_tail():
                    while tail:
                        tail.pop(0)()

                def emit_scores(h, jp):
                    """Scores matmuls + exp + causal mask for one jt pair."""
                    diag = jp >= 2 * ic
                    s = (jp - 2 * ic) * 256 if diag else 0
                    pt = pspair.tile([P, 2, 512], F32, tag="pp")
                    ex = expool.tile([P, 2, 512], FP16, tag="ex")
                    for ko in range(2):
                        jt = 2 * jp + ko
                        sk = s + ko * P if diag else 0
                        nc.tensor.matmul(
                            pt[:, ko, sk:512],
                            lhsT=k_res[:, h, jt * P:(jt + 1) * P],
                            rhs=q_sb[:, h, i0 + sk:i0 + 512],
                            start=True, stop=True)
                    with nc.allow_low_precision(reason="fp16 attn"):
                        if diag:
                            for ko in range(2):
                                jt = 2 * jp + ko
                                sk = s + ko * P
                                nc.scalar.activation(
                                    ex[:, ko, sk:512], pt[:, ko, sk:512],
                                    mybir.ActivationFunctionType.Exp,
                                    scale=SCALE)
                                # causal mask on the 128-wide mixed region:
                                # keep where i0+i-jt*P-p >= 0
                                nc.gpsimd.affine_select(
                                    out=ex[:, ko, sk:sk + P],
                                    in_=ex[:, ko, sk:sk + P],
                                    compare_op=mybir.AluOpType.is_ge,
                                    fill=0.0,
                                    base=i0 + sk - jt * P,
                                    channel_multiplier=-1,
                                    pattern=[[1, P]])
                        else:
                            nc.scalar.activation(
                                ex[:], pt[:],
                                mybir.ActivationFunctionType.Exp, scale=SCALE)
                    return s, ex

                for h in range(H_LOC):
                    dt = denpool.tile([1, 512], F32R, tag="rr")
                    pd_t = None
                    ps_out = psout.tile([P, 512], F32, tag="po")
                    nxt = emit_scores(h, 0)
                    flush_tail()   # previous head's bcast/normalize
                    for jp in range(npair):
                        s, ex = nxt
                        if jp + 1 < npair:
                            nxt = emit_scores(h, jp + 1)
                        if pd_t is None:
                            pd_t = psout.tile([P, 512], F32, tag="po")
                            ps_d = pd_t[0:1, :]
                        diag = jp >= 2 * ic
                        for ko in range(2):
                            jt = 2 * jp + ko
                            sk = s + ko * P if diag else s
                            nc.tensor.matmul(
                                ps_out[:, sk:512],
                                lhsT=v_sb[:, jt, h, :],
                                rhs=ex[:, ko, sk:512],
                                start=(jp == 0 and ko == 0),
                                stop=(jp == npair - 1 and ko == 1),
                                skip_group_check=True)
                            nc.tensor.matmul(
                                ps_d[:, sk:512],
                                lhsT=ones_dn_sb[:],
                                rhs=ex[:, ko, sk:512],
                                start=(jp == 0 and ko == 0),
                                stop=(jp == npair - 1 and ko == 1),
                                skip_group_check=True)
                    with nc.allow_low_precision(reason="softmax recip"):
                        nc.vector.reciprocal(dt[:], ps_d[:])

                    def mk_tail(h=h, dt=dt, pd_t=pd_t, ps_out=ps_out):
                        def run():
                            # broadcast 1/den across partitions via ones
                            # matmul, overwriting the drained den tile
                            ps_b = pd_t[:]
                            nc.tensor.matmul(ps_b, lhsT=ones1[:], rhs=dt[:],
                                             start=True, stop=True)
                            rden = denpool.tile([P, 512], F32R, tag="rden")
                            nc.vector.tensor_copy(out=rden[:], in_=ps_b)
                            with nc.allow_low_precision(reason="fp16 attn out"):
                                nc.vector.tensor_tensor(
                                    at[:, h, :], ps_out[:], rden[:],
                                    mybir.AluOpType.mult)
                                # e4 hi/lo split for the fp8 out-projection
                                # (gpsimd: SBUF-only operands)
                                nc.gpsimd.tensor_copy(out=at_h[:, h, :],
                                                      in_=at[:, h, :])
                                nc.gpsimd.tensor_tensor(
                                    at_l[:, h, :], at[:, h, :], at_h[:, h, :],
                                    mybir.AluOpType.subtract)
                        return run

                    tail.append(mk_tail())
                flush_tail()
                return at_h, at_l, at

            def phase3(ic, ats):
                at_h, at_l = ats[0], ats[1]
                i0 = ic * 512
                for co in range(C // P):
                    if co % 2 == 0:
                        pt = psout.tile([P, 512], F32, tag="po")
                    else:
                        pt = pspair.tile([P, 2, 512], F32, tag="pp")[:, 0, :]
                    cs = slice(co * P, (co + 1) * P)
                    nmm = 0
                    for wo_t, at_ in ((woh_t, at_h), (wol_t, at_h),
                                      (woh_t, at_l)):
                        for kp in range(2):
                            nmm += 1
                            nc.tensor.matmul(
                                pt[:], lhsT=wo_t[kp][:, :, cs],
                                rhs=at_[:, 2 * kp:2 * kp + 2, :],
                                start=(nmm == 1), stop=(nmm == 6),
                                perf_mode=DR, skip_group_check=True)
                    # outT carries the x256 wo scale; host divides it out
                    ot = rpool.tile([P, 512], FP16, tag="ot")
                    with nc.allow_low_precision(reason="fp16 out"):
                        if co % 2 == 0:
                            nc.vector.tensor_copy(out=ot[:], in_=pt[:])
                        else:
                            nc.scalar.copy(out=ot[:], in_=pt[:])
                    nc.sync.dma_start(out=outT[cs, i0:i0 + 512], in_=ot[:])

            for t in range(NT):
                if 1 in phases:
                    phase1(t)
                if 2 in phases:
                    ats = attention(t)
                    if debug:
                        nc.sync.dma_start(out=dbg["at"][t], in_=ats[2][:])
                    if 3 in phases:
                        phase3(t, ats)
            if debug:
                nc.sync.dma_start(out=dbg["q"][:], in_=q_sb[:])
                nc.sync.dma_start(out=dbg["k"][:], in_=k_res[:])
                nc.sync.dma_start(out=dbg["v"][:], in_=v_sb[:])

    nc.finalize()
    return nc


def get_nc(phases=(1, 2, 3)):
    if phases not in _NC_CACHE:
        _NC_CACHE[phases] = _build(phases)
    return _NC_CACHE[phases]


def _rope_tables():
    inv_freq = 1.0 / (BASE ** (np.arange(0, ROT, 2, dtype=np.float64) / ROT))
    freqs = np.arange(T, dtype=np.float64)[:, None] * inv_freq[None, :]  # [T, 32]
    cos_h = np.cos(freqs).T.astype(np.float32)   # [32, T]
    sin_h = np.sin(freqs).T.astype(np.float32)
    cosT = np.concatenate([cos_h, cos_h], axis=0)          # [64, T]
    nsT = np.concatenate([-sin_h, sin_h], axis=0)          # [64, T] signed sin
    return (np.ascontiguousarray(cosT).astype(np.float16),
            np.ascontiguousarray(nsT).astype(np.float16))


def _q8(a):
    return np.clip(a, -240.0, 240.0).astype(ml_dtypes.float8_e4m3)


def _hilo(a):
    hi = _q8(a)
    lo = _q8(np.asarray(a, np.float32) - hi.astype(np.float32))
    return hi, lo


def _bias_pair(b):
    out = np.zeros((1, 2, M), np.float32)
    out[0, 0, :] = b
    return _q8(out)


def make_in_maps(x, Wq, bq, Wk, bk, Wv, bv, Wo, bo):
    cosT, nsT = _rope_tables()
    xh, xl = zip(*[_hilo(np.ascontiguousarray(x[b].T)) for b in range(B)])
    wq_h, wq_l = _hilo(Wq * WSCALE)
    wk_h, wk_l = _hilo(Wk * WSCALE)
    wv_h, wv_l = _hilo(Wv * WSCALE)
    wo_h, wo_l = _hilo(Wo * WSCALE)
    onesx = np.zeros((1, 2, 512), np.float32)
    onesx[0, 0, :] = 1.0
    in_maps = []
    for c in range(N_CORES):
        b, g = divmod(c, TPG)
        ms = slice(g * M, (g + 1) * M)
        in_maps.append({
            "xhT": xh[b],
            "xlT": xl[b],
            "wqh": np.ascontiguousarray(wq_h[ms].T),
            "wql": np.ascontiguousarray(wq_l[ms].T),
            "wkh": np.ascontiguousarray(wk_h[ms].T),
            "wkl": np.ascontiguousarray(wk_l[ms].T),
            "wvh": np.ascontiguousarray(wv_h[ms].T),
            "wvl": np.ascontiguousarray(wv_l[ms].T),
            "woh": np.ascontiguousarray(wo_h[:, ms].T),
            "wol": np.ascontiguousarray(wo_l[:, ms].T),
            "bqc": np.ascontiguousarray(
                bq[ms].reshape(H_LOC, P).T.astype(np.float32)),
            "bkc": np.ascontiguousarray(
                bk[ms].reshape(H_LOC, P).T.astype(np.float32)),
            "bvp": _bias_pair(bv[ms] * WSCALE),
            "onesx": _q8(onesx),
            "ones_dn": np.ones((P, 1), np.float16),
            "ones1": np.full((1, P), 1.0 / WSCALE, np.float32),
            "cosT": cosT,
            "nsT": nsT,
        })
    return in_maps


def assemble(results, bo):
    out = np.empty((B, T, C), dtype=np.float32)
    for b in range(B):
        acc = results[b * TPG]["outT"].astype(np.float32).copy()
        for g in range(1, TPG):
            acc += results[b * TPG + g]["outT"]
        out[b] = acc.T * (1.0 / WSCALE) + bo[None, :]
    return out


def kernel(x, Wq, bq, Wk, bk, Wv, bv, Wo, bo):
    nc = get_nc()
    in_maps = make_in_maps(np.asarray(x, np.float32),
                           np.asarray(Wq, np.float32), np.asarray(bq, np.float32),
                           np.asarray(Wk, np.float32), np.asarray(bk, np.float32),
                           np.asarray(Wv, np.float32), np.asarray(bv, np.float32),
                           np.asarray(Wo, np.float32), np.asarray(bo, np.float32))
    res = run_bass_kernel_spmd(nc, in_maps, list(range(N_CORES)))
    return assemble(res.results, np.asarray(bo, np.float32))


# revision 95
# speedup vs baseline: 1.2701x; 1.0276x over previous
"""Trainium2 Bass kernel for a full causal MHA layer (B=2, T=2048, C=2048, H=16,
partial RoPE on first 64 dims of each 128-dim head).

Sharding over 8 cores: core c handles batch b=c//4 and heads [4g, 4g+4), g=c%4.

v3 design (fp8 hi/lo projections + fp16 attention, fully SBUF-resident):
  - x and Wq/Wk/Wv split host-side into e4m3 hi + e4m3 lo residuals; the three
    projections run as 3-term DoubleRow fp8 chains (hi@hi + lo@hi + hi@lo),
    25% fewer PE cycles than f32r at ~4e-3 relative error.
  - biases folded into each PSUM chain as a 1-partition DoubleRow matmul.
  - q/k path fp16: rot rows evicted to fp16 (DVE), RoPE on DVE (2x mode),
    pass rows evicted on GPSIMD; q_sb/k_res SBUF-resident [128,4,T] fp16.
  - scores/out/den matmuls in fp16 (1 cyc/row); exp on ACT into fp16 pair
    tiles [128,2,512]; causal masking via trimmed affine_selects on GPSIMD.
  - softmax: reciprocal (DVE) -> ones-matmul broadcast into the den pair
    tile's free plane -> copy to SBUF -> fused normalize into fp16 attn.
  - phase 3: fp16 matmuls; result DMA'd straight from PSUM to DRAM (f32).
  - per t-chunk interleave: proj(t) -> attention(ic=t) -> out-proj(ic=t).
Host: slices inputs per core, sums the 4 TP partials per batch, adds bo.
"""

import math

import numpy as np
import ml_dtypes

import concourse.bass as bass
import concourse.mybir as mybir
import concourse.tile as tile
from concourse import bacc
from concourse.bass_utils import run_bass_kernel_spmd

F32 = mybir.dt.float32
F32R = mybir.dt.float32r
FP16 = mybir.dt.float16
E4 = mybir.dt.float8e4
DR = mybir.MatmulPerfMode.DoubleRow

B, T, C = 2, 2048, 2048
H = 16
HS = 128
ROT = 64
HALF = 32
BASE = 10000.0

N_CORES = 8
TPG = 4                # TP group size (heads split)
H_LOC = H // TPG       # 4 heads per core
M = H_LOC * HS         # 512 local head-dim columns
SCALE = 1.0 / math.sqrt(HS)

P = 128
NT = T // 512          # 4 t-chunks of 512
KP = C // 256          # 8 DoubleRow contraction pair-tiles
JT = T // P            # 16 key tiles per head
WSCALE = 256.0         # fp8 weight pre-scale (keeps hi/lo residuals normal)
EXSHIFT = 6.0          # exp bias: keeps e4m3 exp outputs under the 240 max

_NC_CACHE = {}


def _build(phases=(1, 2, 3), debug=False):
    nc = bacc.Bacc(None, target_bir_lowering=False)
    dbg = {}
    if debug:
        dbg["q"] = nc.declare_dram_parameter("qdbg", [P, H_LOC, T], FP16,
                                             isOutput=True)
        dbg["k"] = nc.declare_dram_parameter("kdbg", [P, H_LOC, T], FP16,
                                             isOutput=True)
        dbg["v"] = nc.declare_dram_parameter("vdbg", [P, JT, H_LOC, HS], FP16,
                                             isOutput=True)
        dbg["at"] = nc.declare_dram_parameter("atdbg", [NT, P, H_LOC, 512], FP16,
                                              isOutput=True)

    xhT = nc.declare_dram_parameter("xhT", [C, T], E4, isOutput=False)
    xlT = nc.declare_dram_parameter("xlT", [C, T], E4, isOutput=False)
    w_d = {}
    for w in ("wq", "wk", "wv"):
        for p_ in ("h", "l"):
            w_d[w + p_] = nc.declare_dram_parameter(
                w + p_, [C, M], E4, isOutput=False)
    woh = nc.declare_dram_parameter("woh", [M, C], E4, isOutput=False)
    wol = nc.declare_dram_parameter("wol", [M, C], E4, isOutput=False)
    bqc = nc.declare_dram_parameter("bqc", [P, H_LOC], F32, isOutput=False)
    bkc = nc.declare_dram_parameter("bkc", [P, H_LOC], F32, isOutput=False)
    bvp = nc.declare_dram_parameter("bvp", [1, 2, M], E4, isOutput=False)
    onesx = nc.declare_dram_parameter("onesx", [1, 2, 512], E4, isOutput=False)
    ones_dn = nc.declare_dram_parameter("ones_dn", [P, 1], FP16, isOutput=False)
    ones_dn4 = nc.declare_dram_parameter("ones_dn4", [P, 2, 16], E4, isOutput=False)
    exb = nc.declare_dram_parameter("exb", [P, 1], F32, isOutput=False)
    ones1_d = nc.declare_dram_parameter("ones1", [1, P], F32R, isOutput=False)
    cosT = nc.declare_dram_parameter("cosT", [ROT, T], FP16, isOutput=False)
    nsT = nc.declare_dram_parameter("nsT", [ROT, T], FP16, isOutput=False)
    outT = nc.declare_dram_parameter("outT", [C, T], FP16, isOutput=True)

    # DoubleRow pair views of the fp8 operands: contraction c = kp*256+ko*128+p
    xhpr = xhT[:].rearrange("(kp two p) t -> kp p two t", two=2, p=P)
    xlpr = xlT[:].rearrange("(kp two p) t -> kp p two t", two=2, p=P)
    wpr = {k: v[:].rearrange("(kp two p) m -> kp p two m", two=2, p=P)
           for k, v in w_d.items()}
    wohpr = woh[:].rearrange("(kp two p) c -> kp p two c", two=2, p=P)
    wolpr = wol[:].rearrange("(kp two p) c -> kp p two c", two=2, p=P)

    with tile.TileContext(nc) as tc, \
         tc.tile_pool(name="const", bufs=1) as const:
        cos_sb = const.tile([ROT, T], FP16, tag="cos")
        ns_sb = const.tile([ROT, T], FP16, tag="ns")
        bqc_sb = const.tile([P, H_LOC], F32, tag="bqc")
        bkc_sb = const.tile([P, H_LOC], F32, tag="bkc")
        bvp_sb = const.tile([1, 2, M], E4, tag="bvp")
        onesx_sb = const.tile([1, 2, 512], E4, tag="onesx")
        ones_dn_sb = const.tile([P, 1], FP16, tag="onesdn")
        ones_dn4_sb = const.tile([P, 2, 16], E4, tag="onesdn4")
        exb_sb = const.tile([P, 1], F32, tag="exb")
        ones1 = const.tile([1, P], F32R, tag="ones1")
        k_res = const.tile([P, H_LOC, T], FP16, tag="kres")
        q_sb = const.tile([P, H_LOC, T], FP16, tag="qsb")
        v_sb = const.tile([P, 4, H_LOC, HS], FP16, tag="vsb")
        v_h = const.tile([P, JT, H_LOC, HS], E4, tag="vh")
        v_l = const.tile([P, JT, H_LOC, HS], E4, tag="vl")
        w_t = {k: [const.tile([P, 2, M], E4, name=f"{k}{i}", tag=f"{k}{i}")
                   for i in range(KP)] for k in wpr}
        woh_t = [const.tile([P, 2, C], E4, name=f"woh{i}", tag=f"woh{i}")
                 for i in range(2)]
        wol_t = [const.tile([P, 2, C], E4, name=f"wol{i}", tag=f"wol{i}")
                 for i in range(2)]

        nc.sync.dma_start(out=cos_sb[:], in_=cosT[:])
        nc.sync.dma_start(out=ns_sb[:], in_=nsT[:])
        nc.sync.dma_start(out=bqc_sb[:], in_=bqc[:])
        nc.sync.dma_start(out=bkc_sb[:], in_=bkc[:])
        nc.sync.dma_start(out=bvp_sb[:], in_=bvp[:])
        nc.sync.dma_start(out=onesx_sb[:], in_=onesx[:])
        nc.sync.dma_start(out=ones_dn_sb[:], in_=ones_dn[:])
        nc.sync.dma_start(out=ones_dn4_sb[:], in_=ones_dn4[:])
        nc.sync.dma_start(out=exb_sb[:], in_=exb[:])
        nc.sync.dma_start(out=ones1[:], in_=ones1_d[:])

        with tc.tile_pool(name="px", bufs=24) as xpool, \
             tc.tile_pool(name="rope", bufs=4) as rpool, \
             tc.tile_pool(name="ex", bufs=3) as expool, \
             tc.tile_pool(name="den", bufs=2) as denpool, \
             tc.tile_pool(name="attnp", bufs=2) as apool, \
             tc.tile_pool(name="pair", bufs=3, space="PSUM") as pspair, \
             tc.tile_pool(name="pso", bufs=2, space="PSUM") as psout:

            # startup loads in chain consumption order: per-kp q weights and
            # chunk-0 x tiles interleaved, then k/v weights, wo last; later x
            # chunks drip in during phase1 so they never block shift DMAs
            x_t = {}
            prefetch = []

            def queue_x(tch):
                ts0 = tch * 512
                x_t[tch] = th = {}
                for nm, view in (("h", xhpr), ("l", xlpr)):
                    th[nm] = [xpool.tile([P, 2, 512], E4, tag="x",
                                         name=f"x{nm}{tch}_{i}")
                              for i in range(KP)]
                for kp in range(KP):
                    for nm, view in (("h", xhpr), ("l", xlpr)):
                        prefetch.append((th[nm][kp], view, kp, ts0))

            def drip(n):
                for _ in range(min(n, len(prefetch))):
                    tile_, view, kp, ts0 = prefetch.pop(0)
                    nc.sync.dma_start(out=tile_[:],
                                      in_=view[kp, :, :, ts0:ts0 + 512])

            queue_x(0)
            for kp in range(KP):
                drip(1)
                nc.sync.dma_start(out=w_t["wqh"][kp][:], in_=wpr["wqh"][kp])
                drip(1)
                nc.sync.dma_start(out=w_t["wql"][kp][:], in_=wpr["wql"][kp])
            for kp in range(KP):
                nc.sync.dma_start(out=w_t["wkh"][kp][:], in_=wpr["wkh"][kp])
                nc.sync.dma_start(out=w_t["wkl"][kp][:], in_=wpr["wkl"][kp])
            for kp in range(KP):
                nc.sync.dma_start(out=w_t["wvh"][kp][:], in_=wpr["wvh"][kp])
                nc.sync.dma_start(out=w_t["wvl"][kp][:], in_=wpr["wvl"][kp])
            for i in range(2):
                nc.sync.dma_start(out=woh_t[i][:], in_=wohpr[i])
                nc.sync.dma_start(out=wol_t[i][:], in_=wolpr[i])

            def phase1(tch):
                ts0 = tch * 512
                if tch + 1 < NT and (tch + 1) not in x_t:
                    queue_x(tch + 1)
                xh, xl = x_t[tch]["h"], x_t[tch]["l"]

                for proj, wn, b_sb in (("q", "wq", bqc_sb), ("k", "wk", bkc_sb)):
                    dst = q_sb if proj == "q" else k_res
                    wh, wl = w_t[wn + "h"], w_t[wn + "l"]
                    pt = None
                    for mt in range(H_LOC):
                        if mt % 2 == 0:
                            pt = pspair.tile([P, 2, 512], F32, tag="pp")
                        ps = pt[:, mt % 2, :]
                        ms = slice(mt * P, (mt + 1) * P)
                        nmm = 0
                        for kp in range(KP):
                            for wt_, xt_ in ((wh, xh), (wh, xl), (wl, xh)):
                                nmm += 1
                                nc.tensor.matmul(
                                    ps, lhsT=wt_[kp][:, :, ms], rhs=xt_[kp][:],
                                    start=(kp == 0 and wt_ is wh and xt_ is xh),
                                    stop=(nmm == 3 * KP), perf_mode=DR,
                                    skip_group_check=True)
                        # rot rows 0:64 -> fp16 tmp (bias + 1/WSCALE descale
                        # applied in the eviction), rope on DVE, write dst
                        qtmp = rpool.tile([ROT, 512], FP16, tag="qtmp")
                        with nc.allow_low_precision(reason="fp16 qk path"):
                            nc.scalar.activation(
                                qtmp[:], ps[0:ROT],
                                mybir.ActivationFunctionType.Identity,
                                bias=b_sb[0:ROT, mt:mt + 1], scale=1.0 / WSCALE)
                            # pass rows 64:128 straight to dst (ACT: gpsimd
                            # has no PSUM port)
                            nc.scalar.activation(
                                dst[ROT:P, mt, ts0:ts0 + 512], ps[ROT:P],
                                mybir.ActivationFunctionType.Identity,
                                bias=b_sb[ROT:P, mt:mt + 1], scale=1.0 / WSCALE)
                        qsh = rpool.tile([ROT, 512], FP16, tag="qsh")
                        nc.sync.dma_start(out=qsh[0:HALF], in_=qtmp[HALF:ROT])
                        nc.sync.dma_start(out=qsh[HALF:ROT], in_=qtmp[0:HALF])
                        t1 = rpool.tile([ROT, 512], FP16, tag="t1")
                        nc.vector.tensor_tensor(
                            t1[:], qtmp[:], cos_sb[:, ts0:ts0 + 512],
                            mybir.AluOpType.mult)
                        t2 = rpool.tile([ROT, 512], FP16, tag="t2")
                        nc.vector.tensor_tensor(
                            t2[:], qsh[:], ns_sb[:, ts0:ts0 + 512],
                            mybir.AluOpType.mult)
                        nc.vector.tensor_tensor(
                            dst[0:ROT, mt, ts0:ts0 + 512], t1[:], t2[:],
                            mybir.AluOpType.add)
                        drip(2)

                # v: [t_tile, m] layout
                wh, wl = w_t["wvh"], w_t["wvl"]
                pt = None
                for tt in range(4):
                    if tt % 2 == 0:
                        pt = pspair.tile([P, 2, 512], F32, tag="pp")
                    ps = pt[:, tt % 2, :]
                    ts_ = slice(tt * P, (tt + 1) * P)
                    for kp in range(KP):
                        for xt_, wt_ in ((xh, wh), (xl, wh), (xh, wl)):
                            nc.tensor.matmul(
                                ps, lhsT=xt_[kp][:, :, ts_], rhs=wt_[kp][:],
                                start=(kp == 0 and xt_ is xh and wt_ is wh),
                                stop=False, perf_mode=DR, skip_group_check=True)
                    nc.tensor.matmul(
                        ps, lhsT=onesx_sb[:, :, ts_], rhs=bvp_sb[:],
                        start=False, stop=True, perf_mode=DR,
                        skip_group_check=True)
                    jt = tch * 4 + tt
                    # v: descaled fp16 copy (for diagonal planes) plus an
                    # e4 hi/lo split (for DoubleRow out-matmuls); gpsimd ops
                    # stay SBUF-only
                    with nc.allow_low_precision(reason="fp16 v"):
                        nc.scalar.mul(out=v_sb[:, jt % 4, :, :], in_=ps,
                                      mul=1.0 / WSCALE)
                        nc.gpsimd.tensor_copy(out=v_h[:, jt, :, :],
                                              in_=v_sb[:, jt % 4, :, :])
                        nc.gpsimd.tensor_tensor(
                            v_l[:, jt, :, :], v_sb[:, jt % 4, :, :],
                            v_h[:, jt, :, :], mybir.AluOpType.subtract)
                    drip(2)

            def attention(ic):
                i0 = ic * 512
                npair = 2 * ic + 2
                at_h = apool.tile([P, H_LOC, 512], E4, tag="attnh")
                at_l = apool.tile([P, H_LOC, 512], E4, tag="attnl")
                tail = []

                def flush_tail():
                    while tail:
                        tail.pop(0)()

                def emit_scores(h, jp):
                    """Scores matmuls + shifted exp + causal mask for one jt
                    pair. All exps carry an e^-EXSHIFT factor (cancels in the
                    softmax ratio); non-diagonal pairs quantize to e4m3 so
                    the out/den accumulations can run as DoubleRow fp8."""
                    diag = jp >= 2 * ic
                    s = (jp - 2 * ic) * 256 if diag else 0
                    pt = pspair.tile([P, 2, 512], F32, tag="pp")
                    for ko in range(2):
                        jt = 2 * jp + ko
                        sk = s + ko * P if diag else 0
                        nc.tensor.matmul(
                            pt[:, ko, sk:512],
                            lhsT=k_res[:, h, jt * P:(jt + 1) * P],
                            rhs=q_sb[:, h, i0 + sk:i0 + 512],
                            start=True, stop=True)
                    with nc.allow_low_precision(reason="fp16 attn"):
                        if diag:
                            ex = expool.tile([P, 2, 512], FP16, tag="ex")
                            for ko in range(2):
                                jt = 2 * jp + ko
                                sk = s + ko * P
                                nc.scalar.activation(
                                    ex[:, ko, sk:512], pt[:, ko, sk:512],
                                    mybir.ActivationFunctionType.Exp,
                                    scale=SCALE, bias=exb_sb[:, 0:1])
                                # causal mask on the 128-wide mixed region:
                                # keep where i0+i-jt*P-p >= 0
                                nc.gpsimd.affine_select(
                                    out=ex[:, ko, sk:sk + P],
                                    in_=ex[:, ko, sk:sk + P],
                                    compare_op=mybir.AluOpType.is_ge,
                                    fill=0.0,
                                    base=i0 + sk - jt * P,
                                    channel_multiplier=-1,
                                    pattern=[[1, P]])
                        else:
                            ex = expool.tile([P, 2, 512], E4, tag="ex4")
                            nc.scalar.activation(
                                ex[:], pt[:],
                                mybir.ActivationFunctionType.Exp, scale=SCALE,
                                bias=exb_sb[:, 0:1])
                    return s, ex

                # diagonal pairs first: the head then ends on cheap single-
                # instruction exps, so the next head's out-matmuls are not
                # stuck behind a 4-instruction diagonal exp burst on ACT
                jp_order = list(range(2 * ic, npair)) + list(range(2 * ic))
                for h in range(H_LOC):
                    dt = denpool.tile([1, 512], F32R, tag="rr")
                    pd_t = None
                    ps_out = psout.tile([P, 512], F32, tag="po")
                    pend = [emit_scores(h, jp_order[0])]
                    flush_tail()   # previous head's bcast/normalize
                    if npair > 1:
                        pend.append(emit_scores(h, jp_order[1]))
                    for idx in range(npair):
                        jp = jp_order[idx]
                        s, ex = pend.pop(0)
                        if idx + 2 < npair:
                            pend.append(emit_scores(h, jp_order[idx + 2]))
                        if pd_t is None:
                            pd_t = psout.tile([P, 512], F32, tag="po")
                            ps_d = pd_t[0:1, :]
                        first = idx == 0
                        last = idx == npair - 1
                        diag = jp >= 2 * ic
                        if not diag:
                            # DoubleRow fp8 over the jt pair: 2-term hi/lo v
                            for vt in (v_h, v_l):
                                nc.tensor.matmul(
                                    ps_out[:],
                                    lhsT=vt[:, 2 * jp:2 * jp + 2, h, :],
                                    rhs=ex[:],
                                    start=(first and vt is v_h),
                                    stop=(last and vt is v_l),
                                    perf_mode=DR,
                                    skip_group_check=True)
                            nc.tensor.matmul(
                                ps_d[:], lhsT=ones_dn4_sb[:, :, 0:1], rhs=ex[:],
                                start=first, stop=last, perf_mode=DR,
                                skip_group_check=True)
                        else:
                            for ko in range(2):
                                jt = 2 * jp + ko
                                sk = s + ko * P
                                nc.tensor.matmul(
                                    ps_out[:, sk:512],
                                    lhsT=v_sb[:, jt % 4, h, :],
                                    rhs=ex[:, ko, sk:512],
                                    start=(first and ko == 0),
                                    stop=(last and ko == 1),
                                    skip_group_check=True)
                                nc.tensor.matmul(
                                    ps_d[:, sk:512],
                                    lhsT=ones_dn_sb[:],
                                    rhs=ex[:, ko, sk:512],
                                    start=(first and ko == 0),
                                    stop=(last and ko == 1),
                                    skip_group_check=True)
                    with nc.allow_low_precision(reason="softmax recip"):
                        nc.vector.reciprocal(dt[:], ps_d[:])

                    def mk_tail(h=h, dt=dt, pd_t=pd_t, ps_out=ps_out):
                        def run():
                            # broadcast 1/den across partitions via ones
                            # matmul, overwriting the drained den tile
                            ps_b = pd_t[:]
                            nc.tensor.matmul(ps_b, lhsT=ones1[:], rhs=dt[:],
                                             start=True, stop=True)
                            rden = denpool.tile([P, 512], F32R, tag="rden")
                            nc.vector.tensor_copy(out=rden[:], in_=ps_b)
                            atf = denpool.tile([P, 512], FP16, tag="atf")
                            with nc.allow_low_precision(reason="fp16 attn out"):
                                nc.vector.tensor_tensor(
                                    atf[:], ps_out[:], rden[:],
                                    mybir.AluOpType.mult)
                                # e4 hi/lo split for the fp8 out-projection
                                # (gpsimd: SBUF-only operands)
                                nc.gpsimd.tensor_copy(out=at_h[:, h, :],
                                                      in_=atf[:])
                                nc.gpsimd.tensor_tensor(
                                    at_l[:, h, :], atf[:], at_h[:, h, :],
                                    mybir.AluOpType.subtract)
                        return run

                    tail.append(mk_tail())
                flush_tail()
                return at_h, at_l

            def phase3(ic, ats):
                at_h, at_l = ats[0], ats[1]
                i0 = ic * 512
                for co in range(C // P):
                    ptf = psout.tile([P, 512], F32, tag="po")
                    pt = ptf[:]
                    cs = slice(co * P, (co + 1) * P)
                    nmm = 0
                    for wo_t, at_ in ((woh_t, at_h), (wol_t, at_h),
                                      (woh_t, at_l)):
                        for kp in range(2):
                            nmm += 1
                            nc.tensor.matmul(
                                pt, lhsT=wo_t[kp][:, :, cs],
                                rhs=at_[:, 2 * kp:2 * kp + 2, :],
                                start=(nmm == 1), stop=(nmm == 6),
                                perf_mode=DR, skip_group_check=True)
                    # outT carries the x256 wo scale; host divides it out
                    ot = rpool.tile([P, 512], FP16, tag="ot")
                    with nc.allow_low_precision(reason="fp16 out"):
                        if co % 2 == 0:
                            nc.vector.tensor_copy(out=ot[:], in_=pt)
                        else:
                            nc.scalar.copy(out=ot[:], in_=pt)
                    nc.sync.dma_start(out=outT[cs, i0:i0 + 512], in_=ot[:])

            for t in range(NT):
                if 1 in phases:
                    phase1(t)
                if 2 in phases:
                    ats = attention(t)
                    if 3 in phases:
                        phase3(t, ats)
            if debug:
                nc.sync.dma_start(out=dbg["q"][:], in_=q_sb[:])
                nc.sync.dma_start(out=dbg["k"][:], in_=k_res[:])
                nc.sync.dma_start(out=dbg["v"][:], in_=v_sb[:])

    nc.finalize()
    return nc


def get_nc(phases=(1, 2, 3)):
    if phases not in _NC_CACHE:
        _NC_CACHE[phases] = _build(phases)
    return _NC_CACHE[phases]


def _rope_tables():
    inv_freq = 1.0 / (BASE ** (np.arange(0, ROT, 2, dtype=np.float64) / ROT))
    freqs = np.arange(T, dtype=np.float64)[:, None] * inv_freq[None, :]  # [T, 32]
    cos_h = np.cos(freqs).T.astype(np.float32)   # [32, T]
    sin_h = np.sin(freqs).T.astype(np.float32)
    cosT = np.concatenate([cos_h, cos_h], axis=0)          # [64, T]
    nsT = np.concatenate([-sin_h, sin_h], axis=0)          # [64, T] signed sin
    return (np.ascontiguousarray(cosT).astype(np.float16),
            np.ascontiguousarray(nsT).astype(np.float16))


def _q8(a):
    return np.clip(a, -240.0, 240.0).astype(ml_dtypes.float8_e4m3)


def _hilo(a):
    hi = _q8(a)
    lo = _q8(np.asarray(a, np.float32) - hi.astype(np.float32))
    return hi, lo


def _bias_pair(b):
    out = np.zeros((1, 2, M), np.float32)
    out[0, 0, :] = b
    return _q8(out)


def make_in_maps(x, Wq, bq, Wk, bk, Wv, bv, Wo, bo):
    cosT, nsT = _rope_tables()
    xh, xl = zip(*[_hilo(np.ascontiguousarray(x[b].T)) for b in range(B)])
    wq_h, wq_l = _hilo(Wq * WSCALE)
    wk_h, wk_l = _hilo(Wk * WSCALE)
    wv_h, wv_l = _hilo(Wv * WSCALE)
    wo_h, wo_l = _hilo(Wo * WSCALE)
    onesx = np.zeros((1, 2, 512), np.float32)
    onesx[0, 0, :] = 1.0
    in_maps = []
    for c in range(N_CORES):
        b, g = divmod(c, TPG)
        ms = slice(g * M, (g + 1) * M)
        in_maps.append({
            "xhT": xh[b],
            "xlT": xl[b],
            "wqh": np.ascontiguousarray(wq_h[ms].T),
            "wql": np.ascontiguousarray(wq_l[ms].T),
            "wkh": np.ascontiguousarray(wk_h[ms].T),
            "wkl": np.ascontiguousarray(wk_l[ms].T),
            "wvh": np.ascontiguousarray(wv_h[ms].T),
            "wvl": np.ascontiguousarray(wv_l[ms].T),
            "woh": np.ascontiguousarray(wo_h[:, ms].T),
            "wol": np.ascontiguousarray(wo_l[:, ms].T),
            "bqc": np.ascontiguousarray(
                bq[ms].reshape(H_LOC, P).T.astype(np.float32)),
            "bkc": np.ascontiguousarray(
                bk[ms].reshape(H_LOC, P).T.astype(np.float32)),
            "bvp": _bias_pair(bv[ms] * WSCALE),
            "onesx": _q8(onesx),
            "ones_dn": np.ones((P, 1), np.float16),
            "ones_dn4": np.ones((P, 2, 16), ml_dtypes.float8_e4m3),
            "exb": np.full((P, 1), -EXSHIFT, np.float32),
            "ones1": np.ones((1, P), np.float32),
            "cosT": cosT,
            "nsT": nsT,
        })
    return in_maps


def assemble(results, bo):
    out = np.empty((B, T, C), dtype=np.float32)
    for b in range(B):
        acc = results[b * TPG]["outT"].astype(np.float32).copy()
        for g in range(1, TPG):
            acc += results[b * TPG + g]["outT"]
        out[b] = acc.T * (1.0 / WSCALE) + bo[None, :]
    return out


def kernel(x, Wq, bq, Wk, bk, Wv, bv, Wo, bo):
    nc = get_nc()
    in_maps = make_in_maps(np.asarray(x, np.float32),
                           np.asarray(Wq, np.float32), np.asarray(bq, np.float32),
                           np.asarray(Wk, np.float32), np.asarray(bk, np.float32),
                           np.asarray(Wv, np.float32), np.asarray(bv, np.float32),
                           np.asarray(Wo, np.float32), np.asarray(bo, np.float32))
    res = run_bass_kernel_spmd(nc, in_maps, list(range(N_CORES)))
    return assemble(res.results, np.asarray(bo, np.float32))


# revision 101
# speedup vs baseline: 1.3249x; 1.0432x over previous
"""Trainium2 Bass kernel for a full causal MHA layer (B=2, T=2048, C=2048, H=16,
partial RoPE on first 64 dims of each 128-dim head).

Sharding over 8 cores: core c handles batch b=c//4 and heads [4g, 4g+4), g=c%4.

Design (fp8 hi/lo matmuls + fp16 attention, fully SBUF-resident):
  - x, Wq/Wk/Wv (x256 prescale) and Wo (x256) split host-side into e4m3 hi +
    lo residuals; projections and the out-projection run as 3-term DoubleRow
    fp8 chains (hi@hi + lo@hi + hi@lo), 25% fewer PE cycles than f32r.
  - q/k biases applied in the ACT evictions (with the 1/256 descale); the v
    bias rides in each PSUM chain as a 1-partition DoubleRow matmul.
  - q/k path fp16: rot rows + pass rows evicted on ACT, RoPE on DVE (2x
    mode); q_sb/k_res/v/attn all SBUF-resident - no DRAM scratch.
  - scores in fp16 (1 cyc/row). exp on ACT carries a fixed e^-6 shift
    (cancels in the softmax ratio): non-diagonal jt pairs quantize to e4m3
    pair tiles so out (2-term v hi/lo) and den run as DoubleRow fp8;
    diagonal pairs stay fp16 with per-plane width trimming and
    affine_select causal masks on GPSIMD.
  - per head: diagonal pairs processed first, scores emitted two pairs
    ahead of out/den, and the softmax broadcast/normalize deferred into the
    next head's scores window to keep the PE stream dense.
  - schedule per t-chunk: proj(t) -> attention(ic=t) -> out-proj(ic=t);
    x tiles for chunk t+1 drip two DMAs per chain so the latency-critical
    RoPE shift DMAs are never queued behind a prefetch burst.
Host: slices inputs per core, sums the 4 TP partials per batch (outT carries
the x256 Wo scale, divided out here), adds bo.
"""

import math

import numpy as np
import ml_dtypes

import concourse.bass as bass
import concourse.mybir as mybir
import concourse.tile as tile
from concourse import bacc
from concourse.bass_utils import run_bass_kernel_spmd

F32 = mybir.dt.float32
F32R = mybir.dt.float32r
FP16 = mybir.dt.float16
E4 = mybir.dt.float8e4
DR = mybir.MatmulPerfMode.DoubleRow

B, T, C = 2, 2048, 2048
H = 16
HS = 128
ROT = 64
HALF = 32
BASE = 10000.0

N_CORES = 8
TPG = 4                # TP group size (heads split)
H_LOC = H // TPG       # 4 heads per core
M = H_LOC * HS         # 512 local head-dim columns
SCALE = 1.0 / math.sqrt(HS)

P = 128
NT = T // 512          # 4 t-chunks of 512
KP = C // 256          # 8 DoubleRow contraction pair-tiles
JT = T // P            # 16 key tiles per head
WSCALE = 256.0         # fp8 weight pre-scale (keeps hi/lo residuals normal)
EXSHIFT = 6.0          # exp bias: keeps e4m3 exp outputs under the 240 max

_NC_CACHE = {}


def _build(phases=(1, 2, 3), debug=False):
    nc = bacc.Bacc(None, target_bir_lowering=False)
    dbg = {}
    if debug:
        dbg["q"] = nc.declare_dram_parameter("qdbg", [P, H_LOC, T], FP16,
                                             isOutput=True)
        dbg["k"] = nc.declare_dram_parameter("kdbg", [P, H_LOC, T], FP16,
                                             isOutput=True)
        dbg["v"] = nc.declare_dram_parameter("vdbg", [P, JT, H_LOC, HS], FP16,
                                             isOutput=True)
        dbg["at"] = nc.declare_dram_parameter("atdbg", [NT, P, H_LOC, 512], FP16,
                                              isOutput=True)

    xhT = nc.declare_dram_parameter("xhT", [C, T], E4, isOutput=False)
    xlT = nc.declare_dram_parameter("xlT", [C, T], E4, isOutput=False)
    w_d = {}
    for w in ("wq", "wk", "wv"):
        for p_ in ("h", "l"):
            w_d[w + p_] = nc.declare_dram_parameter(
                w + p_, [C, M], E4, isOutput=False)
    woh = nc.declare_dram_parameter("woh", [M, C], E4, isOutput=False)
    wol = nc.declare_dram_parameter("wol", [M, C], E4, isOutput=False)
    bqc = nc.declare_dram_parameter("bqc", [P, H_LOC], F32, isOutput=False)
    bkc = nc.declare_dram_parameter("bkc", [P, H_LOC], F32, isOutput=False)
    bvp = nc.declare_dram_parameter("bvp", [1, 2, M], E4, isOutput=False)
    onesx = nc.declare_dram_parameter("onesx", [1, 2, 512], E4, isOutput=False)
    ones_dn = nc.declare_dram_parameter("ones_dn", [P, 1], FP16, isOutput=False)
    ones_dn4 = nc.declare_dram_parameter("ones_dn4", [P, 2, 16], E4, isOutput=False)
    exb = nc.declare_dram_parameter("exb", [P, 1], F32, isOutput=False)
    ones1_d = nc.declare_dram_parameter("ones1", [1, P], F32R, isOutput=False)
    cosT = nc.declare_dram_parameter("cosT", [ROT, T], FP16, isOutput=False)
    nsT = nc.declare_dram_parameter("nsT", [ROT, T], FP16, isOutput=False)
    outT = nc.declare_dram_parameter("outT", [C, T], FP16, isOutput=True)

    # DoubleRow pair views of the fp8 operands: contraction c = kp*256+ko*128+p
    xhpr = xhT[:].rearrange("(kp two p) t -> kp p two t", two=2, p=P)
    xlpr = xlT[:].rearrange("(kp two p) t -> kp p two t", two=2, p=P)
    wpr = {k: v[:].rearrange("(kp two p) m -> kp p two m", two=2, p=P)
           for k, v in w_d.items()}
    wohpr = woh[:].rearrange("(kp two p) c -> kp p two c", two=2, p=P)
    wolpr = wol[:].rearrange("(kp two p) c -> kp p two c", two=2, p=P)

    with tile.TileContext(nc) as tc, \
         tc.tile_pool(name="const", bufs=1) as const:
        cos_sb = const.tile([ROT, T], FP16, tag="cos")
        ns_sb = const.tile([ROT, T], FP16, tag="ns")
        bqc_sb = const.tile([P, H_LOC], F32, tag="bqc")
        bkc_sb = const.tile([P, H_LOC], F32, tag="bkc")
        bvp_sb = const.tile([1, 2, M], E4, tag="bvp")
        onesx_sb = const.tile([1, 2, 512], E4, tag="onesx")
        ones_dn_sb = const.tile([P, 1], FP16, tag="onesdn")
        ones_dn4_sb = const.tile([P, 2, 16], E4, tag="onesdn4")
        exb_sb = const.tile([P, 1], F32, tag="exb")
        ones1 = const.tile([1, P], F32R, tag="ones1")
        k_res = const.tile([P, H_LOC, T], FP16, tag="kres")
        q_sb = const.tile([P, H_LOC, T], FP16, tag="qsb")
        v_sb = const.tile([P, 4, H_LOC, HS], FP16, tag="vsb")
        v_h = const.tile([P, JT, H_LOC, HS], E4, tag="vh")
        v_l = const.tile([P, JT, H_LOC, HS], E4, tag="vl")
        w_t = {k: [const.tile([P, 2, M], E4, name=f"{k}{i}", tag=f"{k}{i}")
                   for i in range(KP)] for k in wpr}
        woh_t = [const.tile([P, 2, C], E4, name=f"woh{i}", tag=f"woh{i}")
                 for i in range(2)]
        wol_t = [const.tile([P, 2, C], E4, name=f"wol{i}", tag=f"wol{i}")
                 for i in range(2)]

        nc.sync.dma_start(out=cos_sb[:], in_=cosT[:])
        nc.sync.dma_start(out=ns_sb[:], in_=nsT[:])
        nc.sync.dma_start(out=bqc_sb[:], in_=bqc[:])
        nc.sync.dma_start(out=bkc_sb[:], in_=bkc[:])
        nc.sync.dma_start(out=bvp_sb[:], in_=bvp[:])
        nc.sync.dma_start(out=onesx_sb[:], in_=onesx[:])
        nc.sync.dma_start(out=ones_dn_sb[:], in_=ones_dn[:])
        nc.sync.dma_start(out=ones_dn4_sb[:], in_=ones_dn4[:])
        nc.sync.dma_start(out=exb_sb[:], in_=exb[:])
        nc.sync.dma_start(out=ones1[:], in_=ones1_d[:])

        with tc.tile_pool(name="px", bufs=24) as xpool, \
             tc.tile_pool(name="rope", bufs=4) as rpool, \
             tc.tile_pool(name="ex", bufs=3) as expool, \
             tc.tile_pool(name="den", bufs=2) as denpool, \
             tc.tile_pool(name="attnp", bufs=2) as apool, \
             tc.tile_pool(name="pair", bufs=3, space="PSUM") as pspair, \
             tc.tile_pool(name="pso", bufs=2, space="PSUM") as psout:

            # startup loads in chain consumption order: per-kp q weights and
            # chunk-0 x tiles interleaved, then k/v weights, wo last; later x
            # chunks drip in during phase1 so they never block shift DMAs
            x_t = {}
            prefetch = []

            def queue_x(tch):
                ts0 = tch * 512
                x_t[tch] = th = {}
                for nm, view in (("h", xhpr), ("l", xlpr)):
                    th[nm] = [xpool.tile([P, 2, 512], E4, tag="x",
                                         name=f"x{nm}{tch}_{i}")
                              for i in range(KP)]
                for kp in range(KP):
                    for nm, view in (("h", xhpr), ("l", xlpr)):
                        prefetch.append((th[nm][kp], view, kp, ts0))

            def drip(n):
                for _ in range(min(n, len(prefetch))):
                    tile_, view, kp, ts0 = prefetch.pop(0)
                    nc.sync.dma_start(out=tile_[:],
                                      in_=view[kp, :, :, ts0:ts0 + 512])

            queue_x(0)
            for kp in range(KP):
                drip(1)
                nc.sync.dma_start(out=w_t["wqh"][kp][:], in_=wpr["wqh"][kp])
                drip(1)
                nc.sync.dma_start(out=w_t["wql"][kp][:], in_=wpr["wql"][kp])
            for kp in range(KP):
                nc.sync.dma_start(out=w_t["wkh"][kp][:], in_=wpr["wkh"][kp])
                nc.sync.dma_start(out=w_t["wkl"][kp][:], in_=wpr["wkl"][kp])
            for kp in range(KP):
                nc.sync.dma_start(out=w_t["wvh"][kp][:], in_=wpr["wvh"][kp])
                nc.sync.dma_start(out=w_t["wvl"][kp][:], in_=wpr["wvl"][kp])
            for i in range(2):
                nc.sync.dma_start(out=woh_t[i][:], in_=wohpr[i])
                nc.sync.dma_start(out=wol_t[i][:], in_=wolpr[i])

            def phase1(tch):
                ts0 = tch * 512
                if tch + 1 < NT and (tch + 1) not in x_t:
                    queue_x(tch + 1)
                xh, xl = x_t[tch]["h"], x_t[tch]["l"]

                for proj, wn, b_sb in (("q", "wq", bqc_sb), ("k", "wk", bkc_sb)):
                    dst = q_sb if proj == "q" else k_res
                    wh, wl = w_t[wn + "h"], w_t[wn + "l"]
                    pt = None
                    for mt in range(H_LOC):
                        if mt % 2 == 0:
                            pt = pspair.tile([P, 2, 512], F32, tag="pp")
                        ps = pt[:, mt % 2, :]
                        ms = slice(mt * P, (mt + 1) * P)
                        nmm = 0
                        for kp in range(KP):
                            for wt_, xt_ in ((wh, xh), (wh, xl), (wl, xh)):
                                nmm += 1
                                nc.tensor.matmul(
                                    ps, lhsT=wt_[kp][:, :, ms], rhs=xt_[kp][:],
                                    start=(kp == 0 and wt_ is wh and xt_ is xh),
                                    stop=(nmm == 3 * KP), perf_mode=DR,
                                    skip_group_check=True)
                        # rot rows 0:64 -> fp16 tmp (bias + 1/WSCALE descale
                        # applied in the eviction), rope on DVE, write dst
                        qtmp = rpool.tile([ROT, 512], FP16, tag="qtmp")
                        with nc.allow_low_precision(reason="fp16 qk path"):
                            nc.scalar.activation(
                                qtmp[:], ps[0:ROT],
                                mybir.ActivationFunctionType.Identity,
                                bias=b_sb[0:ROT, mt:mt + 1], scale=1.0 / WSCALE)
                            # pass rows 64:128 straight to dst (ACT: gpsimd
                            # has no PSUM port)
                            nc.scalar.activation(
                                dst[ROT:P, mt, ts0:ts0 + 512], ps[ROT:P],
                                mybir.ActivationFunctionType.Identity,
                                bias=b_sb[ROT:P, mt:mt + 1], scale=1.0 / WSCALE)
                        qsh = rpool.tile([ROT, 512], FP16, tag="qsh")
                        nc.sync.dma_start(out=qsh[0:HALF], in_=qtmp[HALF:ROT])
                        nc.sync.dma_start(out=qsh[HALF:ROT], in_=qtmp[0:HALF])
                        t1 = rpool.tile([ROT, 512], FP16, tag="t1")
                        nc.vector.tensor_tensor(
                            t1[:], qtmp[:], cos_sb[:, ts0:ts0 + 512],
                            mybir.AluOpType.mult)
                        t2 = rpool.tile([ROT, 512], FP16, tag="t2")
                        nc.vector.tensor_tensor(
                            t2[:], qsh[:], ns_sb[:, ts0:ts0 + 512],
                            mybir.AluOpType.mult)
                        nc.vector.tensor_tensor(
                            dst[0:ROT, mt, ts0:ts0 + 512], t1[:], t2[:],
                            mybir.AluOpType.add)
                        drip(2)

                # v: [t_tile, m] layout
                wh, wl = w_t["wvh"], w_t["wvl"]
                pt = None
                for tt in range(4):
                    if tt % 2 == 0:
                        pt = pspair.tile([P, 2, 512], F32, tag="pp")
                    ps = pt[:, tt % 2, :]
                    ts_ = slice(tt * P, (tt + 1) * P)
                    for kp in range(KP):
                        for xt_, wt_ in ((xh, wh), (xl, wh), (xh, wl)):
                            nc.tensor.matmul(
                                ps, lhsT=xt_[kp][:, :, ts_], rhs=wt_[kp][:],
                                start=(kp == 0 and xt_ is xh and wt_ is wh),
                                stop=False, perf_mode=DR, skip_group_check=True)
                    nc.tensor.matmul(
                        ps, lhsT=onesx_sb[:, :, ts_], rhs=bvp_sb[:],
                        start=False, stop=True, perf_mode=DR,
                        skip_group_check=True)
                    jt = tch * 4 + tt
                    # v: descaled fp16 copy (for diagonal planes) plus an
                    # e4 hi/lo split (for DoubleRow out-matmuls); gpsimd ops
                    # stay SBUF-only
                    with nc.allow_low_precision(reason="fp16 v"):
                        nc.scalar.mul(out=v_sb[:, jt % 4, :, :], in_=ps,
                                      mul=1.0 / WSCALE)
                        nc.gpsimd.tensor_copy(out=v_h[:, jt, :, :],
                                              in_=v_sb[:, jt % 4, :, :])
                        nc.gpsimd.tensor_tensor(
                            v_l[:, jt, :, :], v_sb[:, jt % 4, :, :],
                            v_h[:, jt, :, :], mybir.AluOpType.subtract)
                    drip(2)

            def attention(ic):
                i0 = ic * 512
                npair = 2 * ic + 2
                at_h = apool.tile([P, H_LOC, 512], E4, tag="attnh")
                at_l = apool.tile([P, H_LOC, 512], E4, tag="attnl")
                tail = []

                def flush_tail():
                    while tail:
                        tail.pop(0)()

                def emit_scores(h, jp):
                    """Scores matmuls + shifted exp + causal mask for one jt
                    pair. All exps carry an e^-EXSHIFT factor (cancels in the
                    softmax ratio); non-diagonal pairs quantize to e4m3 so
                    the out/den accumulations can run as DoubleRow fp8."""
                    diag = jp >= 2 * ic
                    s = (jp - 2 * ic) * 256 if diag else 0
                    pt = pspair.tile([P, 2, 512], F32, tag="pp")
                    for ko in range(2):
                        jt = 2 * jp + ko
                        sk = s + ko * P if diag else 0
                        nc.tensor.matmul(
                            pt[:, ko, sk:512],
                            lhsT=k_res[:, h, jt * P:(jt + 1) * P],
                            rhs=q_sb[:, h, i0 + sk:i0 + 512],
                            start=True, stop=True)
                    with nc.allow_low_precision(reason="fp16 attn"):
                        if diag:
                            ex = expool.tile([P, 2, 512], FP16, tag="ex")
                            for ko in range(2):
                                jt = 2 * jp + ko
                                sk = s + ko * P
                                nc.scalar.activation(
                                    ex[:, ko, sk:512], pt[:, ko, sk:512],
                                    mybir.ActivationFunctionType.Exp,
                                    scale=SCALE, bias=exb_sb[:, 0:1])
                                # causal mask on the 128-wide mixed region:
                                # keep where i0+i-jt*P-p >= 0
                                nc.gpsimd.affine_select(
                                    out=ex[:, ko, sk:sk + P],
                                    in_=ex[:, ko, sk:sk + P],
                                    compare_op=mybir.AluOpType.is_ge,
                                    fill=0.0,
                                    base=i0 + sk - jt * P,
                                    channel_multiplier=-1,
                                    pattern=[[1, P]])
                        else:
                            ex = expool.tile([P, 2, 512], E4, tag="ex4")
                            nc.scalar.activation(
                                ex[:], pt[:],
                                mybir.ActivationFunctionType.Exp, scale=SCALE,
                                bias=exb_sb[:, 0:1])
                    return s, ex

                # diagonal pairs first: the head then ends on cheap single-
                # instruction exps, so the next head's out-matmuls are not
                # stuck behind a 4-instruction diagonal exp burst on ACT
                jp_order = list(range(2 * ic, npair)) + list(range(2 * ic))
                for h in range(H_LOC):
                    dt = denpool.tile([1, 512], F32R, tag="rr")
                    pd_t = None
                    ps_out = psout.tile([P, 512], F32, tag="po")
                    pend = [emit_scores(h, jp_order[0])]
                    flush_tail()   # previous head's bcast/normalize
                    if npair > 1:
                        pend.append(emit_scores(h, jp_order[1]))
                    for idx in range(npair):
                        jp = jp_order[idx]
                        s, ex = pend.pop(0)
                        if idx + 2 < npair:
                            pend.append(emit_scores(h, jp_order[idx + 2]))
                        if pd_t is None:
                            pd_t = psout.tile([P, 512], F32, tag="po")
                            ps_d = pd_t[0:1, :]
                        first = idx == 0
                        last = idx == npair - 1
                        diag = jp >= 2 * ic
                        if not diag:
                            # DoubleRow fp8 over the jt pair: 2-term hi/lo v
                            for vt in (v_h, v_l):
                                nc.tensor.matmul(
                                    ps_out[:],
                                    lhsT=vt[:, 2 * jp:2 * jp + 2, h, :],
                                    rhs=ex[:],
                                    start=(first and vt is v_h),
                                    stop=(last and vt is v_l),
                                    perf_mode=DR,
                                    skip_group_check=True)
                            nc.tensor.matmul(
                                ps_d[:], lhsT=ones_dn4_sb[:, :, 0:1], rhs=ex[:],
                                start=first, stop=last, perf_mode=DR,
                                skip_group_check=True)
                        else:
                            for ko in range(2):
                                jt = 2 * jp + ko
                                sk = s + ko * P
                                nc.tensor.matmul(
                                    ps_out[:, sk:512],
                                    lhsT=v_sb[:, jt % 4, h, :],
                                    rhs=ex[:, ko, sk:512],
                                    start=(first and ko == 0),
                                    stop=(last and ko == 1),
                                    skip_group_check=True)
                                nc.tensor.matmul(
                                    ps_d[:, sk:512],
                                    lhsT=ones_dn_sb[:],
                                    rhs=ex[:, ko, sk:512],
                                    start=(first and ko == 0),
                                    stop=(last and ko == 1),
                                    skip_group_check=True)
                    with nc.allow_low_precision(reason="softmax recip"):
                        nc.vector.reciprocal(dt[:], ps_d[:])

                    def mk_tail(h=h, dt=dt, pd_t=pd_t, ps_out=ps_out):
                        def run():
                            # broadcast 1/den across partitions via ones
                            # matmul, overwriting the drained den tile
                            ps_b = pd_t[:]
                            nc.tensor.matmul(ps_b, lhsT=ones1[:], rhs=dt[:],
                                             start=True, stop=True)
                            rden = denpool.tile([P, 512], F32R, tag="rden")
                            nc.vector.tensor_copy(out=rden[:], in_=ps_b)
                            atf = denpool.tile([P, 512], FP16, tag="atf")
                            with nc.allow_low_precision(reason="fp16 attn out"):
                                nc.vector.tensor_tensor(
                                    atf[:], ps_out[:], rden[:],
                                    mybir.AluOpType.mult)
                                # e4 hi/lo split for the fp8 out-projection
                                # (gpsimd: SBUF-only operands)
                                nc.gpsimd.tensor_copy(out=at_h[:, h, :],
                                                      in_=atf[:])
                                nc.gpsimd.tensor_tensor(
                                    at_l[:, h, :], atf[:], at_h[:, h, :],
                                    mybir.AluOpType.subtract)
                        return run

                    tail.append(mk_tail())
                flush_tail()
                return at_h, at_l

            def phase3(ic, ats):
                at_h, at_l = ats[0], ats[1]
                i0 = ic * 512
                for co in range(C // P):
                    ptf = psout.tile([P, 512], F32, tag="po")
                    pt = ptf[:]
                    cs = slice(co * P, (co + 1) * P)
                    nmm = 0
                    for wo_t, at_ in ((woh_t, at_h), (wol_t, at_h),
                                      (woh_t, at_l)):
                        for kp in range(2):
                            nmm += 1
                            nc.tensor.matmul(
                                pt, lhsT=wo_t[kp][:, :, cs],
                                rhs=at_[:, 2 * kp:2 * kp + 2, :],
                                start=(nmm == 1), stop=(nmm == 6),
                                perf_mode=DR, skip_group_check=True)
                    # outT carries the x256 wo scale; host divides it out
                    ot = rpool.tile([P, 512], FP16, tag="ot")
                    with nc.allow_low_precision(reason="fp16 out"):
                        nc.vector.tensor_copy(out=ot[:], in_=pt)
                    nc.sync.dma_start(out=outT[cs, i0:i0 + 512], in_=ot[:])

            for t in range(NT):
                if 1 in phases:
                    phase1(t)
                if 2 in phases:
                    ats = attention(t)
                    if 3 in phases:
                        phase3(t, ats)
            if debug:
                nc.sync.dma_start(out=dbg["q"][:], in_=q_sb[:])
                nc.sync.dma_start(out=dbg["k"][:], in_=k_res[:])
                nc.sync.dma_start(out=dbg["v"][:], in_=v_sb[:])

    nc.finalize()
    return nc


def get_nc(phases=(1, 2, 3)):
    if phases not in _NC_CACHE:
        _NC_CACHE[phases] = _build(phases)
    return _NC_CACHE[phases]


def _rope_tables():
    inv_freq = 1.0 / (BASE ** (np.arange(0, ROT, 2, dtype=np.float64) / ROT))
    freqs = np.arange(T, dtype=np.float64)[:, None] * inv_freq[None, :]  # [T, 32]
    cos_h = np.cos(freqs).T.astype(np.float32)   # [32, T]
    sin_h = np.sin(freqs).T.astype(np.float32)
    cosT = np.concatenate([cos_h, cos_h], axis=0)          # [64, T]
    nsT = np.concatenate([-sin_h, sin_h], axis=0)          # [64, T] signed sin
    return (np.ascontiguousarray(cosT).astype(np.float16),
            np.ascontiguousarray(nsT).astype(np.float16))


def _q8(a):
    return np.clip(a, -240.0, 240.0).astype(ml_dtypes.float8_e4m3)


def _hilo(a):
    hi = _q8(a)
    lo = _q8(np.asarray(a, np.float32) - hi.astype(np.float32))
    return hi, lo


def _bias_pair(b):
    out = np.zeros((1, 2, M), np.float32)
    out[0, 0, :] = b
    return _q8(out)


def make_in_maps(x, Wq, bq, Wk, bk, Wv, bv, Wo, bo):
    cosT, nsT = _rope_tables()
    xh, xl = zip(*[_hilo(np.ascontiguousarray(x[b].T)) for b in range(B)])
    wq_h, wq_l = _hilo(Wq * WSCALE)
    wk_h, wk_l = _hilo(Wk * WSCALE)
    wv_h, wv_l = _hilo(Wv * WSCALE)
    wo_h, wo_l = _hilo(Wo * WSCALE)
    onesx = np.zeros((1, 2, 512), np.float32)
    onesx[0, 0, :] = 1.0
    in_maps = []
    for c in range(N_CORES):
        b, g = divmod(c, TPG)
        ms = slice(g * M, (g + 1) * M)
        in_maps.append({
            "xhT": xh[b],
            "xlT": xl[b],
            "wqh": np.ascontiguousarray(wq_h[ms].T),
            "wql": np.ascontiguousarray(wq_l[ms].T),
            "wkh": np.ascontiguousarray(wk_h[ms].T),
            "wkl": np.ascontiguousarray(wk_l[ms].T),
            "wvh": np.ascontiguousarray(wv_h[ms].T),
            "wvl": np.ascontiguousarray(wv_l[ms].T),
            "woh": np.ascontiguousarray(wo_h[:, ms].T),
            "wol": np.ascontiguousarray(wo_l[:, ms].T),
            "bqc": np.ascontiguousarray(
                bq[ms].reshape(H_LOC, P).T.astype(np.float32)),
            "bkc": np.ascontiguousarray(
                bk[ms].reshape(H_LOC, P).T.astype(np.float32)),
            "bvp": _bias_pair(bv[ms] * WSCALE),
            "onesx": _q8(onesx),
            "ones_dn": np.ones((P, 1), np.float16),
            "ones_dn4": np.ones((P, 2, 16), ml_dtypes.float8_e4m3),
            "exb": np.full((P, 1), -EXSHIFT, np.float32),
            "ones1": np.ones((1, P), np.float32),
            "cosT": cosT,
            "nsT": nsT,
        })
    return in_maps


def assemble(results, bo):
    out = np.empty((B, T, C), dtype=np.float32)
    for b in range(B):
        acc = results[b * TPG]["outT"].astype(np.float32).copy()
        for g in range(1, TPG):
            acc += results[b * TPG + g]["outT"]
        out[b] = acc.T * (1.0 / WSCALE) + bo[None, :]
    return out


def kernel(x, Wq, bq, Wk, bk, Wv, bv, Wo, bo):
    nc = get_nc()
    in_maps = make_in_maps(np.asarray(x, np.float32),
                           np.asarray(Wq, np.float32), np.asarray(bq, np.float32),
                           np.asarray(Wk, np.float32), np.asarray(bk, np.float32),
                           np.asarray(Wv, np.float32), np.asarray(bv, np.float32),
                           np.asarray(Wo, np.float32), np.asarray(bo, np.float32))
    res = run_bass_kernel_spmd(nc, in_maps, list(range(N_CORES)))
    return assemble(res.results, np.asarray(bo, np.float32))


# revision 112
# speedup vs baseline: 1.3543x; 1.0222x over previous
"""Trainium2 Bass kernel for a full causal MHA layer (B=2, T=2048, C=2048, H=16,
partial RoPE on first 64 dims of each 128-dim head).

Sharding over 8 cores: core c handles batch b=c//4 and heads [4g, 4g+4), g=c%4.

Design (fp8 hi/lo matmuls + fp16 attention, fully SBUF-resident):
  - x, Wq/Wk/Wv (x256 prescale) and Wo (x256) split host-side into e4m3 hi +
    lo residuals; projections and the out-projection run as 3-term DoubleRow
    fp8 chains (hi@hi + lo@hi + hi@lo), 25% fewer PE cycles than f32r.
  - q/k biases applied in the ACT evictions (with the 1/256 descale); the v
    bias rides in each PSUM chain as a 1-partition DoubleRow matmul.
  - q/k path fp16: rot rows + pass rows evicted on ACT, RoPE on DVE (2x
    mode); q_sb/k_res/v/attn all SBUF-resident - no DRAM scratch.
  - scores in fp16 (1 cyc/row). exp on ACT carries a fixed e^-6 shift
    (cancels in the softmax ratio): non-diagonal jt pairs quantize to e4m3
    pair tiles so out (2-term v hi/lo) and den run as DoubleRow fp8;
    diagonal pairs stay fp16 with per-plane width trimming and
    affine_select causal masks on GPSIMD.
  - per head: diagonal pairs processed first, scores emitted two pairs
    ahead of out/den, and the softmax broadcast/normalize deferred into the
    next head's scores window to keep the PE stream dense.
  - schedule per t-chunk: proj(t) -> attention(ic=t) -> out-proj(ic=t);
    x tiles for chunk t+1 drip two DMAs per chain so the latency-critical
    RoPE shift DMAs are never queued behind a prefetch burst.
Host: slices inputs per core, sums the 4 TP partials per batch (outT carries
the x256 Wo scale, divided out here), adds bo.
"""

import math

import numpy as np
import ml_dtypes

import concourse.bass as bass
import concourse.mybir as mybir
import concourse.tile as tile
from concourse import bacc
from concourse.bass_utils import run_bass_kernel_spmd

F32 = mybir.dt.float32
F32R = mybir.dt.float32r
FP16 = mybir.dt.float16
E4 = mybir.dt.float8e4
DR = mybir.MatmulPerfMode.DoubleRow

B, T, C = 2, 2048, 2048
H = 16
HS = 128
ROT = 64
HALF = 32
BASE = 10000.0

N_CORES = 8
TPG = 4                # TP group size (heads split)
H_LOC = H // TPG       # 4 heads per core
M = H_LOC * HS         # 512 local head-dim columns
SCALE = 1.0 / math.sqrt(HS)

P = 128
NT = T // 512          # 4 t-chunks of 512
KP = C // 256          # 8 DoubleRow contraction pair-tiles
JT = T // P            # 16 key tiles per head
WSCALE = 256.0         # fp8 weight pre-scale (keeps hi/lo residuals normal)
EXSHIFT = 6.0          # exp bias: keeps e4m3 exp outputs under the 240 max

_NC_CACHE = {}


def _build(phases=(1, 2, 3), debug=False):
    nc = bacc.Bacc(None, target_bir_lowering=False)
    dbg = {}
    if debug:
        dbg["q"] = nc.declare_dram_parameter("qdbg", [P, H_LOC, T], FP16,
                                             isOutput=True)
        dbg["k"] = nc.declare_dram_parameter("kdbg", [P, H_LOC, T], FP16,
                                             isOutput=True)
        dbg["v"] = nc.declare_dram_parameter("vdbg", [P, JT, H_LOC, HS], FP16,
                                             isOutput=True)
        dbg["at"] = nc.declare_dram_parameter("atdbg", [NT, P, H_LOC, 512], FP16,
                                              isOutput=True)

    xhT = nc.declare_dram_parameter("xhT", [C, T], E4, isOutput=False)
    xlT = nc.declare_dram_parameter("xlT", [C, T], E4, isOutput=False)
    w_d = {}
    for w in ("wq", "wk", "wv"):
        for p_ in ("h", "l"):
            w_d[w + p_] = nc.declare_dram_parameter(
                w + p_, [C, M], E4, isOutput=False)
    woh = nc.declare_dram_parameter("woh", [M, C], E4, isOutput=False)
    wol = nc.declare_dram_parameter("wol", [M, C], E4, isOutput=False)
    bqc = nc.declare_dram_parameter("bqc", [P, H_LOC], F32, isOutput=False)
    bkc = nc.declare_dram_parameter("bkc", [P, H_LOC], F32, isOutput=False)
    bvp = nc.declare_dram_parameter("bvp", [1, 2, M], E4, isOutput=False)
    onesx = nc.declare_dram_parameter("onesx", [1, 2, 512], E4, isOutput=False)
    ones_dn = nc.declare_dram_parameter("ones_dn", [P, 1], FP16, isOutput=False)
    ones_dn4 = nc.declare_dram_parameter("ones_dn4", [P, 2, 16], E4, isOutput=False)
    exb = nc.declare_dram_parameter("exb", [P, 1], F32, isOutput=False)
    ones1_d = nc.declare_dram_parameter("ones1", [1, P], F32R, isOutput=False)
    cosT = nc.declare_dram_parameter("cosT", [ROT, T], FP16, isOutput=False)
    nsT = nc.declare_dram_parameter("nsT", [ROT, T], FP16, isOutput=False)
    outT = nc.declare_dram_parameter("outT", [C, T], FP16, isOutput=True)

    # DoubleRow pair views of the fp8 operands: contraction c = kp*256+ko*128+p
    xhpr = xhT[:].rearrange("(kp two p) t -> kp p two t", two=2, p=P)
    xlpr = xlT[:].rearrange("(kp two p) t -> kp p two t", two=2, p=P)
    wpr = {k: v[:].rearrange("(kp two p) m -> kp p two m", two=2, p=P)
           for k, v in w_d.items()}
    wohpr = woh[:].rearrange("(kp two p) c -> kp p two c", two=2, p=P)
    wolpr = wol[:].rearrange("(kp two p) c -> kp p two c", two=2, p=P)

    with tile.TileContext(nc) as tc, \
         tc.tile_pool(name="const", bufs=1) as const:
        cos_sb = const.tile([ROT, T], FP16, tag="cos")
        ns_sb = const.tile([ROT, T], FP16, tag="ns")
        bqc_sb = const.tile([P, H_LOC], F32, tag="bqc")
        bkc_sb = const.tile([P, H_LOC], F32, tag="bkc")
        bvp_sb = const.tile([1, 2, M], E4, tag="bvp")
        onesx_sb = const.tile([1, 2, 512], E4, tag="onesx")
        ones_dn_sb = const.tile([P, 1], FP16, tag="onesdn")
        ones_dn4_sb = const.tile([P, 2, 16], E4, tag="onesdn4")
        exb_sb = const.tile([P, 1], F32, tag="exb")
        ones1 = const.tile([1, P], F32R, tag="ones1")
        k_res = const.tile([P, H_LOC, T], FP16, tag="kres")
        q_sb = const.tile([P, H_LOC, T], FP16, tag="qsb")
        v_sb = const.tile([P, 4, H_LOC, HS], FP16, tag="vsb")
        v_h = const.tile([P, JT, H_LOC, HS], E4, tag="vh")
        v_l = const.tile([P, JT, H_LOC, HS], E4, tag="vl")
        w_t = {k: [const.tile([P, 2, M], E4, name=f"{k}{i}", tag=f"{k}{i}")
                   for i in range(KP)] for k in wpr}
        woh_t = [const.tile([P, 2, C], E4, name=f"woh{i}", tag=f"woh{i}")
                 for i in range(2)]
        wol_t = [const.tile([P, 2, C], E4, name=f"wol{i}", tag=f"wol{i}")
                 for i in range(2)]

        nc.sync.dma_start(out=cos_sb[:], in_=cosT[:])
        nc.sync.dma_start(out=ns_sb[:], in_=nsT[:])
        nc.sync.dma_start(out=bqc_sb[:], in_=bqc[:])
        nc.sync.dma_start(out=bkc_sb[:], in_=bkc[:])
        nc.sync.dma_start(out=bvp_sb[:], in_=bvp[:])
        nc.sync.dma_start(out=onesx_sb[:], in_=onesx[:])
        nc.sync.dma_start(out=ones_dn_sb[:], in_=ones_dn[:])
        nc.sync.dma_start(out=ones_dn4_sb[:], in_=ones_dn4[:])
        nc.sync.dma_start(out=exb_sb[:], in_=exb[:])
        nc.sync.dma_start(out=ones1[:], in_=ones1_d[:])

        with tc.tile_pool(name="px", bufs=24) as xpool, \
             tc.tile_pool(name="rope", bufs=4) as rpool, \
             tc.tile_pool(name="ex", bufs=3) as expool, \
             tc.tile_pool(name="den", bufs=2) as denpool, \
             tc.tile_pool(name="attnp", bufs=2) as apool, \
             tc.tile_pool(name="pair", bufs=3, space="PSUM") as pspair, \
             tc.tile_pool(name="pso", bufs=2, space="PSUM") as psout:

            # startup loads in chain consumption order: per-kp q weights and
            # chunk-0 x tiles interleaved, then k/v weights, wo last; later x
            # chunks drip in during phase1 so they never block shift DMAs
            x_t = {}
            prefetch = []

            def queue_x(tch):
                ts0 = tch * 512
                x_t[tch] = th = {}
                for nm, view in (("h", xhpr), ("l", xlpr)):
                    th[nm] = [xpool.tile([P, 2, 512], E4, tag="x",
                                         name=f"x{nm}{tch}_{i}")
                              for i in range(KP)]
                for nm, view in (("h", xhpr), ("l", xlpr)):
                    for kp in range(KP):
                        prefetch.append((th[nm][kp], view, kp, ts0))

            def drip(n):
                for _ in range(min(n, len(prefetch))):
                    tile_, view, kp, ts0 = prefetch.pop(0)
                    nc.sync.dma_start(out=tile_[:],
                                      in_=view[kp, :, :, ts0:ts0 + 512])

            queue_x(0)
            for kp in range(KP):
                drip(1)
                nc.sync.dma_start(out=w_t["wqh"][kp][:], in_=wpr["wqh"][kp])
            for kp in range(KP):
                drip(1)
                nc.sync.dma_start(out=w_t["wql"][kp][:], in_=wpr["wql"][kp])
            for kp in range(KP):
                nc.sync.dma_start(out=w_t["wkh"][kp][:], in_=wpr["wkh"][kp])
                nc.sync.dma_start(out=w_t["wkl"][kp][:], in_=wpr["wkl"][kp])
            for kp in range(KP):
                nc.sync.dma_start(out=w_t["wvh"][kp][:], in_=wpr["wvh"][kp])
                nc.sync.dma_start(out=w_t["wvl"][kp][:], in_=wpr["wvl"][kp])
            for i in range(2):
                nc.sync.dma_start(out=woh_t[i][:], in_=wohpr[i])
                nc.sync.dma_start(out=wol_t[i][:], in_=wolpr[i])

            def phase1(tch):
                ts0 = tch * 512
                if tch + 1 < NT and (tch + 1) not in x_t:
                    queue_x(tch + 1)
                xh, xl = x_t[tch]["h"], x_t[tch]["l"]

                for proj, wn, b_sb in (("q", "wq", bqc_sb), ("k", "wk", bkc_sb)):
                    dst = q_sb if proj == "q" else k_res
                    wh, wl = w_t[wn + "h"], w_t[wn + "l"]
                    pt = None
                    for mt in range(H_LOC):
                        if mt % 2 == 0:
                            pt = pspair.tile([P, 2, 512], F32, tag="pp")
                        ps = pt[:, mt % 2, :]
                        ms = slice(mt * P, (mt + 1) * P)
                        nmm = 0
                        for wt_, xt_ in ((wh, xh), (wh, xl), (wl, xh)):
                            for kp in range(KP):
                                nmm += 1
                                nc.tensor.matmul(
                                    ps, lhsT=wt_[kp][:, :, ms], rhs=xt_[kp][:],
                                    start=(kp == 0 and wt_ is wh and xt_ is xh),
                                    stop=(nmm == 3 * KP), perf_mode=DR,
                                    skip_group_check=True)
                        # rot rows 0:64 -> fp16 tmp (bias + 1/WSCALE descale
                        # applied in the eviction), rope on DVE, write dst
                        qtmp = rpool.tile([ROT, 512], FP16, tag="qtmp")
                        with nc.allow_low_precision(reason="fp16 qk path"):
                            nc.scalar.activation(
                                qtmp[:], ps[0:ROT],
                                mybir.ActivationFunctionType.Identity,
                                bias=b_sb[0:ROT, mt:mt + 1], scale=1.0 / WSCALE)
                            # pass rows 64:128 straight to dst (ACT: gpsimd
                            # has no PSUM port)
                            nc.scalar.activation(
                                dst[ROT:P, mt, ts0:ts0 + 512], ps[ROT:P],
                                mybir.ActivationFunctionType.Identity,
                                bias=b_sb[ROT:P, mt:mt + 1], scale=1.0 / WSCALE)
                        qsh = rpool.tile([ROT, 512], FP16, tag="qsh")
                        nc.sync.dma_start(out=qsh[0:HALF], in_=qtmp[HALF:ROT])
                        nc.sync.dma_start(out=qsh[HALF:ROT], in_=qtmp[0:HALF])
                        t1 = rpool.tile([ROT, 512], FP16, tag="t1")
                        nc.vector.tensor_tensor(
                            t1[:], qtmp[:], cos_sb[:, ts0:ts0 + 512],
                            mybir.AluOpType.mult)
                        t2 = rpool.tile([ROT, 512], FP16, tag="t2")
                        nc.vector.tensor_tensor(
                            t2[:], qsh[:], ns_sb[:, ts0:ts0 + 512],
                            mybir.AluOpType.mult)
                        nc.vector.tensor_tensor(
                            dst[0:ROT, mt, ts0:ts0 + 512], t1[:], t2[:],
                            mybir.AluOpType.add)
                        drip(2)

                # v: [t_tile, m] layout
                wh, wl = w_t["wvh"], w_t["wvl"]
                pt = None
                for tt in range(4):
                    if tt % 2 == 0:
                        pt = pspair.tile([P, 2, 512], F32, tag="pp")
                    ps = pt[:, tt % 2, :]
                    ts_ = slice(tt * P, (tt + 1) * P)
                    for xt_, wt_ in ((xh, wh), (xl, wh), (xh, wl)):
                        for kp in range(KP):
                            nc.tensor.matmul(
                                ps, lhsT=xt_[kp][:, :, ts_], rhs=wt_[kp][:],
                                start=(kp == 0 and xt_ is xh and wt_ is wh),
                                stop=False, perf_mode=DR, skip_group_check=True)
                    nc.tensor.matmul(
                        ps, lhsT=onesx_sb[:, :, ts_], rhs=bvp_sb[:],
                        start=False, stop=True, perf_mode=DR,
                        skip_group_check=True)
                    jt = tch * 4 + tt
                    # v: descaled fp16 copy (for diagonal planes) plus an
                    # e4 hi/lo split (for DoubleRow out-matmuls); gpsimd ops
                    # stay SBUF-only
                    with nc.allow_low_precision(reason="fp16 v"):
                        nc.scalar.mul(out=v_sb[:, jt % 4, :, :], in_=ps,
                                      mul=1.0 / WSCALE)
                        nc.gpsimd.tensor_copy(out=v_h[:, jt, :, :],
                                              in_=v_sb[:, jt % 4, :, :])
                        nc.gpsimd.tensor_tensor(
                            v_l[:, jt, :, :], v_sb[:, jt % 4, :, :],
                            v_h[:, jt, :, :], mybir.AluOpType.subtract)
                    drip(2)

            def attention(ic):
                i0 = ic * 512
                npair = 2 * ic + 2
                at_h = apool.tile([P, H_LOC, 512], E4, tag="attnh")
                at_l = apool.tile([P, H_LOC, 512], E4, tag="attnl")
                tail = []

                def flush_tail():
                    while tail:
                        tail.pop(0)()

                def emit_scores(h, jp):
                    """Scores matmuls + shifted exp + causal mask for one jt
                    pair. All exps carry an e^-EXSHIFT factor (cancels in the
                    softmax ratio); non-diagonal pairs quantize to e4m3 so
                    the out/den accumulations can run as DoubleRow fp8."""
                    diag = jp >= 2 * ic
                    s = (jp - 2 * ic) * 256 if diag else 0
                    pt = pspair.tile([P, 2, 512], F32, tag="pp")
                    for ko in range(2):
                        jt = 2 * jp + ko
                        sk = s + ko * P if diag else 0
                        nc.tensor.matmul(
                            pt[:, ko, sk:512],
                            lhsT=k_res[:, h, jt * P:(jt + 1) * P],
                            rhs=q_sb[:, h, i0 + sk:i0 + 512],
                            start=True, stop=True)
                    with nc.allow_low_precision(reason="fp16 attn"):
                        if diag:
                            ex = expool.tile([P, 2, 512], FP16, tag="ex")
                            for ko in range(2):
                                jt = 2 * jp + ko
                                sk = s + ko * P
                                nc.scalar.activation(
                                    ex[:, ko, sk:512], pt[:, ko, sk:512],
                                    mybir.ActivationFunctionType.Exp,
                                    scale=SCALE, bias=exb_sb[:, 0:1])
                                # causal mask on the 128-wide mixed region:
                                # keep where i0+i-jt*P-p >= 0
                                nc.gpsimd.affine_select(
                                    out=ex[:, ko, sk:sk + P],
                                    in_=ex[:, ko, sk:sk + P],
                                    compare_op=mybir.AluOpType.is_ge,
                                    fill=0.0,
                                    base=i0 + sk - jt * P,
                                    channel_multiplier=-1,
                                    pattern=[[1, P]])
                        else:
                            ex = expool.tile([P, 2, 512], E4, tag="ex4")
                            nc.scalar.activation(
                                ex[:], pt[:],
                                mybir.ActivationFunctionType.Exp, scale=SCALE,
                                bias=exb_sb[:, 0:1])
                    return s, ex

                # diagonal pairs first: the head then ends on cheap single-
                # instruction exps, so the next head's out-matmuls are not
                # stuck behind a 4-instruction diagonal exp burst on ACT
                jp_order = list(range(2 * ic, npair)) + list(range(2 * ic))
                for h in range(H_LOC):
                    dt = denpool.tile([1, 512], F32R, tag="rr")
                    pd_t = None
                    ps_out = psout.tile([P, 512], F32, tag="po")
                    pend = [emit_scores(h, jp_order[0])]
                    flush_tail()   # previous head's bcast/normalize
                    if npair > 1:
                        pend.append(emit_scores(h, jp_order[1]))
                    for idx in range(npair):
                        jp = jp_order[idx]
                        s, ex = pend.pop(0)
                        if idx + 2 < npair:
                            pend.append(emit_scores(h, jp_order[idx + 2]))
                        if pd_t is None:
                            pd_t = psout.tile([P, 512], F32, tag="po")
                            ps_d = pd_t[0:1, :]
                        first = idx == 0
                        last = idx == npair - 1
                        diag = jp >= 2 * ic
                        if not diag:
                            # DoubleRow fp8 over the jt pair: 2-term hi/lo v
                            for vt in (v_h, v_l):
                                nc.tensor.matmul(
                                    ps_out[:],
                                    lhsT=vt[:, 2 * jp:2 * jp + 2, h, :],
                                    rhs=ex[:],
                                    start=(first and vt is v_h),
                                    stop=(last and vt is v_l),
                                    perf_mode=DR,
                                    skip_group_check=True)
                            nc.tensor.matmul(
                                ps_d[:], lhsT=ones_dn4_sb[:, :, 0:1], rhs=ex[:],
                                start=first, stop=last, perf_mode=DR,
                                skip_group_check=True)
                        else:
                            for ko in range(2):
                                jt = 2 * jp + ko
                                sk = s + ko * P
                                nc.tensor.matmul(
                                    ps_out[:, sk:512],
                                    lhsT=v_sb[:, jt % 4, h, :],
                                    rhs=ex[:, ko, sk:512],
                                    start=(first and ko == 0),
                                    stop=(last and ko == 1),
                                    skip_group_check=True)
                                nc.tensor.matmul(
                                    ps_d[:, sk:512],
                                    lhsT=ones_dn_sb[:],
                                    rhs=ex[:, ko, sk:512],
                                    start=(first and ko == 0),
                                    stop=(last and ko == 1),
                                    skip_group_check=True)
                    with nc.allow_low_precision(reason="softmax recip"):
                        nc.vector.reciprocal(dt[:], ps_d[:])

                    def mk_tail(h=h, dt=dt, pd_t=pd_t, ps_out=ps_out):
                        def run():
                            # broadcast 1/den across partitions via ones
                            # matmul, overwriting the drained den tile
                            ps_b = pd_t[:]
                            nc.tensor.matmul(ps_b, lhsT=ones1[:], rhs=dt[:],
                                             start=True, stop=True)
                            rden = denpool.tile([P, 512], F32R, tag="rden")
                            nc.vector.tensor_copy(out=rden[:], in_=ps_b)
                            atf = denpool.tile([P, 512], FP16, tag="atf")
                            with nc.allow_low_precision(reason="fp16 attn out"):
                                nc.vector.tensor_tensor(
                                    atf[:], ps_out[:], rden[:],
                                    mybir.AluOpType.mult)
                                # e4 hi/lo split for the fp8 out-projection
                                # (gpsimd: SBUF-only operands)
                                nc.gpsimd.tensor_copy(out=at_h[:, h, :],
                                                      in_=atf[:])
                                nc.gpsimd.tensor_tensor(
                                    at_l[:, h, :], atf[:], at_h[:, h, :],
                                    mybir.AluOpType.subtract)
                        return run

                    tail.append(mk_tail())
                flush_tail()
                return at_h, at_l

            def phase3(ic, ats):
                at_h, at_l = ats[0], ats[1]
                i0 = ic * 512
                for co in range(C // P):
                    ptf = psout.tile([P, 512], F32, tag="po")
                    pt = ptf[:]
                    cs = slice(co * P, (co + 1) * P)
                    nmm = 0
                    for wo_t, at_ in ((woh_t, at_h), (wol_t, at_h),
                                      (woh_t, at_l)):
                        for kp in range(2):
                            nmm += 1
                            nc.tensor.matmul(
                                pt, lhsT=wo_t[kp][:, :, cs],
                                rhs=at_[:, 2 * kp:2 * kp + 2, :],
                                start=(nmm == 1), stop=(nmm == 6),
                                perf_mode=DR, skip_group_check=True)
                    # outT carries the x256 wo scale; host divides it out
                    ot = otpool.tile([P, 512], FP16, tag="ot")
                    with nc.allow_low_precision(reason="fp16 out"):
                        nc.vector.tensor_copy(out=ot[:], in_=pt)
                    nc.sync.dma_start(out=outT[cs, i0:i0 + 512], in_=ot[:])

            for t in range(NT):
                if 1 in phases:
                    phase1(t)
                if 2 in phases:
                    ats = attention(t)
                    if 3 in phases:
                        phase3(t, ats)
            if debug:
                nc.sync.dma_start(out=dbg["q"][:], in_=q_sb[:])
                nc.sync.dma_start(out=dbg["k"][:], in_=k_res[:])
                nc.sync.dma_start(out=dbg["v"][:], in_=v_sb[:])

    nc.finalize()
    return nc


def get_nc(phases=(1, 2, 3)):
    if phases not in _NC_CACHE:
        _NC_CACHE[phases] = _build(phases)
    return _NC_CACHE[phases]


def _rope_tables():
    inv_freq = 1.0 / (BASE ** (np.arange(0, ROT, 2, dtype=np.float64) / ROT))
    freqs = np.arange(T, dtype=np.float64)[:, None] * inv_freq[None, :]  # [T, 32]
    cos_h = np.cos(freqs).T.astype(np.float32)   # [32, T]
    sin_h = np.sin(freqs).T.astype(np.float32)
    cosT = np.concatenate([cos_h, cos_h], axis=0)          # [64, T]
    nsT = np.concatenate([-sin_h, sin_h], axis=0)          # [64, T] signed sin
    return (np.ascontiguousarray(cosT).astype(np.float16),
            np.ascontiguousarray(nsT).astype(np.float16))


def _q8(a):
    return np.clip(a, -240.0, 240.0).astype(ml_dtypes.float8_e4m3)


def _hilo(a):
    hi = _q8(a)
    lo = _q8(np.asarray(a, np.float32) - hi.astype(np.float32))
    return hi, lo


def _bias_pair(b):
    out = np.zeros((1, 2, M), np.float32)
    out[0, 0, :] = b
    return _q8(out)


def make_in_maps(x, Wq, bq, Wk, bk, Wv, bv, Wo, bo):
    cosT, nsT = _rope_tables()
    xh, xl = zip(*[_hilo(np.ascontiguousarray(x[b].T)) for b in range(B)])
    wq_h, wq_l = _hilo(Wq * WSCALE)
    wk_h, wk_l = _hilo(Wk * WSCALE)
    wv_h, wv_l = _hilo(Wv * WSCALE)
    wo_h, wo_l = _hilo(Wo * WSCALE)
    onesx = np.zeros((1, 2, 512), np.float32)
    onesx[0, 0, :] = 1.0
    in_maps = []
    for c in range(N_CORES):
        b, g = divmod(c, TPG)
        ms = slice(g * M, (g + 1) * M)
        in_maps.append({
            "xhT": xh[b],
            "xlT": xl[b],
            "wqh": np.ascontiguousarray(wq_h[ms].T),
            "wql": np.ascontiguousarray(wq_l[ms].T),
            "wkh": np.ascontiguousarray(wk_h[ms].T),
            "wkl": np.ascontiguousarray(wk_l[ms].T),
            "wvh": np.ascontiguousarray(wv_h[ms].T),
            "wvl": np.ascontiguousarray(wv_l[ms].T),
            "woh": np.ascontiguousarray(wo_h[:, ms].T),
            "wol": np.ascontiguousarray(wo_l[:, ms].T),
            "bqc": np.ascontiguousarray(
                bq[ms].reshape(H_LOC, P).T.astype(np.float32)),
            "bkc": np.ascontiguousarray(
                bk[ms].reshape(H_LOC, P).T.astype(np.float32)),
            "bvp": _bias_pair(bv[ms] * WSCALE),
            "onesx": _q8(onesx),
            "ones_dn": np.ones((P, 1), np.float16),
            "ones_dn4": np.ones((P, 2, 16), ml_dtypes.float8_e4m3),
            "exb": np.full((P, 1), -EXSHIFT, np.float32),
            "ones1": np.ones((1, P), np.float32),
            "cosT": cosT,
            "nsT": nsT,
        })
    return in_maps


def assemble(results, bo):
    out = np.empty((B, T, C), dtype=np.float32)
    for b in range(B):
        acc = results[b * TPG]["outT"].astype(np.float32).copy()
        for g in range(1, TPG):
            acc += results[b * TPG + g]["outT"]
        out[b] = acc.T * (1.0 / WSCALE) + bo[None, :]
    return out


def kernel(x, Wq, bq, Wk, bk, Wv, bv, Wo, bo):
    nc = get_nc()
    in_maps = make_in_maps(np.asarray(x, np.float32),
                           np.asarray(Wq, np.float32), np.asarray(bq, np.float32),
                           np.asarray(Wk, np.float32), np.asarray(bk, np.float32),
                           np.asarray(Wv, np.float32), np.asarray(bv, np.float32),
                           np.asarray(Wo, np.float32), np.asarray(bo, np.float32))
    res = run_bass_kernel_spmd(nc, in_maps, list(range(N_CORES)))
    return assemble(res.results, np.asarray(bo, np.float32))
